# revision 1
# baseline (speedup 1.0000x reference)
# GAT (graph attention) Trainium2 kernel — 8-core row-parallel SPMD.
#
# Math (per head h, rows I owned by a core):
#   h = x @ W_h ; f1 = h@a1 ; f2 = h@a2 ; z_ij = f1_i + f2_j
#   P_ij = adj_ij ? exp(lrelu(z)) : exp(9e-15 ~= 0) ; att = softmax_j(P)
#   out = elu( (P @ h) / (P @ 1) )
# Device factorization (avoids O(N^2) transcendentals):
#   exp(lrelu(z)) = u'_i * v'_j * max(r_i * w_j, 1)
#     r = e^{0.8 f1}, w = e^{0.8 f2}, u' = e^{0.2 f1}, v' = e^{0.2 f2}
#   E2[j,i] = m^T[j,i] * max(r_i * (w_j v'_j), v'_j)     (ts_dual + tt mult)
#   numer[d,i] = u'_i * ([h|1]^T E2)[d,i] + S[d] - (h^T m^T)[d,i]
#   denom[i]   = u'_i * Y1[i] + N - deg_i
# Mask path: SWDGE cast int32->bf16 to DRAM scratch (with free deg row-sums),
# then xbar DMA-transpose loads [j-part, i-free] tiles.

import numpy as np

N = 8192
EMB = 128
HID = 64
NH = 4
NCORES = 8
NB = N // NCORES  # 1024 rows per core

_cache = {}


def build(n=N, nb=NB):
    import concourse.bass as bass
    import concourse.bacc as bacc
    import concourse.tile as tile
    import concourse.mybir as mybir
    from concourse.masks import make_identity

    fp32 = mybir.dt.float32
    bf16 = mybir.dt.bfloat16
    i32 = mybir.dt.int32
    Alu = mybir.AluOpType
    Act = mybir.ActivationFunctionType
    MS = bass.MemorySpace

    nbh = nb // 2           # i-half size
    njc = n // 128          # j chunks
    nic = nb // 128         # i chunks (local rows)
    nsub = nbh // 128       # i subchunks per half
    nxc = n // 128          # x chunks

    nc = bacc.Bacc()
    x_d = nc.declare_dram_parameter("x", [n, EMB], fp32, isOutput=False)
    xb_d = nc.declare_dram_parameter("x_blk", [nb, EMB], fp32, isOutput=False)
    adj_d = nc.declare_dram_parameter("adj_blk", [nb, n], i32, isOutput=False)
    W_d = nc.declare_dram_parameter("W", [NH, EMB, HID], fp32, isOutput=False)
    a_d = nc.declare_dram_parameter("a", [NH, 2 * HID, 1], fp32, isOutput=False)
    out_d = nc.declare_dram_parameter("out_blk", [nb, NH * HID], fp32, isOutput=True)

    with tile.TileContext(nc) as tc:
        with (
            tc.tile_pool(name="const", bufs=1) as const,
            tc.tile_pool(name="ld", bufs=3) as ld,
            tc.tile_pool(name="anat", bufs=2) as anat,
            tc.tile_pool(name="dramp", bufs=1, space=MS.DRAM) as dramp,
            tc.tile_pool(name="mtp", bufs=6) as mtp,
            tc.tile_pool(name="dep", bufs=8) as dep,
            tc.tile_pool(name="esb", bufs=3) as esb,
        ):
            # ---------------- constants ----------------
            ident = const.tile([128, 128], fp32, name="ident", tag="ident")
            make_identity(nc, ident)
            ones_row = const.tile([1, 128], fp32, name="ones_row", tag="ones_row")
            nc.vector.memset(ones_row, 1.0)

            madj0 = dramp.tile([nbh, n], bf16, name="madj0", tag="madj0")
            madj1 = dramp.tile([nbh, n], bf16, name="madj1", tag="madj1")
            madj = [madj0, madj1]
            deg_sb = const.tile([128, nic], fp32, name="deg_sb", tag="deg_sb")

            # ------------- stage A: mask cast (+ deg) -------------
            def stageA(ihalf):
                for ics in range(nic // 2):
                    r0 = ihalf * nbh + ics * 128
                    icg = ihalf * (nic // 2) + ics
                    mnat = anat.tile([128, n], bf16, name="mnat", tag="mnat")
                    nc.gpsimd.dma_start(out=mnat, in_=adj_d[r0:r0 + 128, :])
                    nc.vector.tensor_scalar(
                        out=mnat, in0=mnat, scalar1=1.0, scalar2=None,
                        op0=Alu.mult, op1=Alu.add,
                        accum_out=deg_sb[:, icg:icg + 1])
                    nc.sync.dma_start(
                        out=madj[ihalf][ics * 128:(ics + 1) * 128, :], in_=mnat)

            stageA(0)

            # ---------------- prologue ----------------
            ppsum = tc.alloc_tile_pool(name="ppsum", bufs=2, space=MS.PSUM)
            Wsb = const.tile([128, NH, HID], fp32, name="Wsb", tag="Wsb")
            nc.sync.dma_start(out=Wsb, in_=W_d.rearrange("h e d -> e h d"))
            asb = const.tile([HID, NH, 2], fp32, name="asb", tag="asb")
            nc.sync.dma_start(
                out=asb, in_=a_d.rearrange("h (t d) o -> d h (t o)", t=2))

            # x^T  [128e, n]
            xT = const.tile([128, n], fp32, name="xT", tag="xT")
            for ic in range(nxc):
                xt_nat = ld.tile([128, EMB], fp32, name="xt_nat", tag="xt_nat")
                nc.sync.dma_start(out=xt_nat, in_=x_d[ic * 128:(ic + 1) * 128, :])
                ps = ppsum.tile([128, 128], fp32, name="ps", tag="ps")
                nc.tensor.matmul(ps, xt_nat, ident)
                nc.scalar.copy(out=xT[:, ic * 128:(ic + 1) * 128], in_=ps)
            # x_blk^T [128e, nb]
            xbT = const.tile([128, nb], fp32, name="xbT", tag="xbT")
            for ic in range(nic):
                xb_nat = ld.tile([128, EMB], fp32, name="xb_nat", tag="xt_nat")
                nc.sync.dma_start(out=xb_nat, in_=xb_d[ic * 128:(ic + 1) * 128, :])
                ps = ppsum.tile([128, 128], fp32, name="ps", tag="ps")
                nc.tensor.matmul(ps, xb_nat, ident)
                nc.scalar.copy(out=xbT[:, ic * 128:(ic + 1) * 128], in_=ps)

            # xsum[e] = sum_j x[j,e]
            xsum = const.tile([128, 1], fp32, name="xsum", tag="xsum")
            nc.vector.tensor_reduce(xsum, xT, mybir.AxisListType.X, Alu.add)

            # W^T per head; q = [W a1 | W a2] -> Qsb [128e, NH, 2]
            WTsb = const.tile([HID, NH, 128], fp32, name="WTsb", tag="WTsb")
            Qsb = const.tile([128, NH, 2], fp32, name="Qsb", tag="Qsb")
            for h in range(NH):
                wt_ps = ppsum.tile([HID, 128], fp32, name="wt_ps", tag="ps")
                nc.tensor.matmul(wt_ps, Wsb[:, h, :], ident)
                nc.scalar.copy(out=WTsb[:, h, :], in_=wt_ps)
                q_ps = ppsum.tile([128, 2], fp32, name="q_ps", tag="ps")
                nc.tensor.matmul(q_ps, WTsb[:, h, :], asb[:, h, :])
                nc.scalar.copy(out=Qsb[:, h, :], in_=q_ps)

            Qflat = Qsb.rearrange("p h t -> p (h t)")
            Wflat = Wsb.rearrange("e h d -> e (h d)")

            # f columns for all j: Fcol[p, jc, (h t)] = f_{t,h}[jc*128+p]
            Fcol = const.tile([128, njc, 2 * NH], fp32, name="Fcol", tag="Fcol")
            for jc in range(njc):
                f_ps = ppsum.tile([128, 2 * NH], fp32, name="f_ps", tag="ps")
                nc.tensor.matmul(f_ps, xT[:, jc * 128:(jc + 1) * 128], Qflat)
                nc.scalar.copy(out=Fcol[:, jc, :], in_=f_ps)

            # f rows for local block: Frow [8, nb]
            Frow = const.tile([2 * NH, nb], fp32, name="Frow", tag="Frow")
            for half in range(2):
                fr_ps = ppsum.tile([2 * NH, nbh], fp32, name="fr_ps", tag="ps")
                nc.tensor.matmul(fr_ps, Qflat, xbT[:, half * nbh:(half + 1) * nbh])
                nc.scalar.copy(out=Frow[:, half * nbh:(half + 1) * nbh], in_=fr_ps)

            # FrowT [128, nic, 8]
            FrowT = const.tile([128, nic, 2 * NH], fp32, name="FrowT", tag="FrowT")
            for g in range(nic):
                ft_ps = ppsum.tile([128, 2 * NH], fp32, name="ft_ps", tag="ps")
                nc.tensor.matmul(
                    ft_ps, Frow[:, g * 128:(g + 1) * 128],
                    ident[0:2 * NH, 0:2 * NH])
                nc.scalar.copy(out=FrowT[:, g, :], in_=ft_ps)

            # scalar cols (j side): ETc = e^{f2} (= w v'), Vc = e^{0.2 f2}
            ETc = const.tile([128, njc, NH], fp32, name="ETc", tag="ETc")
            Vc = const.tile([128, njc, NH], fp32, name="Vc", tag="Vc")
            for h in range(NH):
                nc.scalar.activation(ETc[:, :, h], Fcol[:, :, 2 * h + 1], Act.Exp)
                nc.scalar.activation(
                    Vc[:, :, h], Fcol[:, :, 2 * h + 1], Act.Exp, scale=0.2)

            # row side: R8 = e^{0.8 Frow}; U'T = e^{0.2 FrowT}
            R8 = const.tile([2 * NH, nb], fp32, name="R8", tag="R8")
            nc.scalar.activation(R8, Frow, Act.Exp, scale=0.8)
            UpT = const.tile([128, nic, 2 * NH], fp32, name="UpT", tag="UpT")
            nc.scalar.activation(
                UpT.rearrange("p a b -> p (a b)"),
                FrowT.rearrange("p a b -> p (a b)"), Act.Exp, scale=0.2)

            # r broadcast per head [128, nb] bf16: bounce rows via DRAM, then
            # broadcast-load with stride-0 partition AP (+ cast) via SWDGE.
            r8_dram = dramp.tile([2 * NH, nb], fp32, name="r8_dram", tag="r8d")
            nc.sync.dma_start(out=r8_dram, in_=R8)
            rbc = []
            for h in range(NH):
                t = const.tile([128, nb], bf16, name=f"rbc{h}", tag=f"rbc{h}")
                srow = r8_dram[2 * h:2 * h + 1, :]
                src_b = bass.AP(
                    tensor=srow.tensor, offset=srow.offset,
                    ap=[[0, 128]] + [list(d) for d in srow.ap[1:]])
                nc.gpsimd.dma_start(out=t, in_=src_b)
                rbc.append(t)

            # H~ [128, njc, NH, HID+1] bf16 (ones col at [.., HID]) for the
            # per-head X passes, plus a contiguous pair layout for hm passes
            # (matmul weights APs must have a single free dimension).
            Hsb = const.tile([128, njc, NH, HID + 1], bf16, name="Hsb", tag="Hsb")
            Hpair = const.tile([128, njc, NH * HID], bf16, name="Hpair", tag="Hpair")
            nc.vector.memset(Hsb[:, :, :, HID], 1.0)
            for jc in range(njc):
                h_ps = ppsum.tile([128, NH, HID], fp32, name="h_ps", tag="ps")
                nc.tensor.matmul(
                    h_ps.rearrange("p h d -> p (h d)"),
                    xT[:, jc * 128:(jc + 1) * 128], Wflat)
                nc.scalar.copy(out=Hsb[:, jc, :, 0:HID], in_=h_ps)
                nc.scalar.copy(
                    out=Hpair[:, jc, :].rearrange("p (h d) -> p h d", h=NH),
                    in_=h_ps)

            # S row then per-head broadcast [128, HID]
            s_ps = ppsum.tile([1, NH * HID], fp32, name="s_ps", tag="ps")
            nc.tensor.matmul(s_ps, xsum, Wflat)
            S_row = const.tile([1, NH * HID], fp32, name="S_row", tag="S_row")
            nc.scalar.copy(out=S_row, in_=s_ps)
            Sb = []
            for h in range(NH):
                sb_ps = ppsum.tile([128, HID], fp32, name="sb_ps", tag="ps")
                nc.tensor.matmul(sb_ps, ones_row, S_row[:, h * HID:(h + 1) * HID])
                t = const.tile([128, HID], fp32, name=f"Sb{h}", tag=f"Sb{h}")
                nc.scalar.copy(out=t, in_=sb_ps)
                Sb.append(t)

            ppsum.release()

            # ------------- stage A part 2, then degbar -------------
            stageA(1)
            degbar = const.tile([128, nic], fp32, name="degbar", tag="degbar")
            nc.vector.tensor_scalar(
                out=degbar, in0=deg_sb, scalar1=-1.0, scalar2=float(n),
                op0=Alu.mult, op1=Alu.add)

            # ---------------- main loop ----------------
            for ihalf in range(2):
                with (
                    tc.tile_pool(name=f"mm{ihalf}", bufs=1, space=MS.PSUM) as mm,
                    tc.tile_pool(name=f"ep{ihalf}", bufs=2, space=MS.PSUM) as ep,
                ):
                    X = [mm.tile([HID + 1, nbh], fp32, name=f"X{h}", tag=f"X{h}")
                         for h in range(NH)]
                    HM = [mm.tile([128, nbh], fp32, name=f"HM{p}", tag=f"HM{p}")
                          for p in range(2)]
                    for jc in range(njc):
                        mT = mtp.tile([128, nbh], bf16, name="mT", tag="mT")
                        nc.sync.dma_start_transpose(
                            out=mT,
                            in_=madj[ihalf][:, jc * 128:(jc + 1) * 128])
                        for h in range(NH):
                            D2 = dep.tile([128, nbh], bf16, name="D2", tag="D2")
                            nc.vector.tensor_scalar(
                                out=D2,
                                in0=rbc[h][:, ihalf * nbh:(ihalf + 1) * nbh],
                                scalar1=ETc[:, jc, h:h + 1],
                                scalar2=Vc[:, jc, h:h + 1],
                                op0=Alu.mult, op1=Alu.max)
                            E2 = dep.tile([128, nbh], bf16, name="E2", tag="E2")
                            eng_tt = nc.gpsimd if h >= 2 else nc.vector
                            eng_tt.tensor_mul(E2, mT, D2)
                            nc.tensor.matmul(
                                X[h], Hsb[:, jc, h, :], E2,
                                start=(jc == 0), stop=(jc == njc - 1))
                        for p in range(2):
                            nc.tensor.matmul(
                                HM[p],
                                Hpair[:, jc, 128 * p:128 * (p + 1)], mT,
                                start=(jc == 0), stop=(jc == njc - 1))

                    # ---------------- epilogue for this half ----------------
                    XS = []
                    for h in range(NH):
                        t = esb.tile([HID + 1, nbh], fp32,
                                     name=f"XS{h}", tag=f"XS{h}", bufs=1)
                        nc.scalar.copy(out=t, in_=X[h])
                        XS.append(t)
                    HMS = []
                    for p in range(2):
                        t = esb.tile([128, nbh], fp32,
                                     name=f"HMS{p}", tag=f"HMS{p}", bufs=1)
                        nc.scalar.copy(out=t, in_=HM[p])
                        HMS.append(t)

                    for isub in range(nsub):
                        g = ihalf * nsub + isub
                        sl = slice(isub * 128, (isub + 1) * 128)
                        hmT = []
                        for p in range(2):
                            tp = ep.tile([128, 128], fp32, name="tp", tag="tp")
                            nc.tensor.matmul(tp, HMS[p][:, sl], ident)
                            t = esb.tile([128, 128], fp32,
                                         name=f"hmT{p}", tag=f"hmT{p}", bufs=2)
                            nc.scalar.copy(out=t, in_=tp)
                            hmT.append(t)
                        out_tile = esb.tile([128, NH * HID], fp32,
                                            name="out_tile", tag="otile", bufs=2)
                        for h in range(NH):
                            tp = ep.tile([128, HID + 1], fp32, name="tpx", tag="tp")
                            nc.tensor.matmul(
                                tp, XS[h][:, sl], ident[0:HID + 1, 0:HID + 1])
                            XT = esb.tile([128, HID + 1], fp32, name="XT", tag="XT")
                            nc.scalar.copy(out=XT, in_=tp)
                            upc = UpT[:, g, 2 * h:2 * h + 1]
                            n1 = esb.tile([128, HID], fp32, name="n1", tag="n1")
                            nc.vector.tensor_scalar(
                                out=n1, in0=XT[:, 0:HID], scalar1=upc,
                                scalar2=None, op0=Alu.mult)
                            n2 = esb.tile([128, HID], fp32, name="n2", tag="n2")
                            nc.vector.scalar_tensor_tensor(
                                out=n2,
                                in0=hmT[h // 2][:, (h % 2) * HID:
                                                (h % 2) * HID + HID],
                                scalar=-1.0, in1=n1, op0=Alu.mult, op1=Alu.add)
                            n3 = esb.tile([128, HID], fp32, name="n3", tag="n3")
                            nc.vector.tensor_add(n3, n2, Sb[h])
                            dcol = esb.tile([128, 1], fp32, name="dcol", tag="dcol")
                            nc.vector.tensor_scalar(
                                out=dcol, in0=XT[:, HID:HID + 1], scalar1=upc,
                                scalar2=degbar[:, g:g + 1],
                                op0=Alu.mult, op1=Alu.add)
                            rec = esb.tile([128, 1], fp32, name="rec", tag="rec")
                            nc.vector.reciprocal(rec, dcol)
                            smT = esb.tile([128, HID], fp32, name="smT", tag="smT")
                            nc.vector.tensor_scalar(
                                out=smT, in0=n3, scalar1=rec, scalar2=None,
                                op0=Alu.mult)
                            # elu = (max(sm,0)-1) + exp(min(sm,0))
                            ea = esb.tile([128, HID], fp32, name="ea", tag="ea")
                            nc.vector.tensor_scalar_min(ea, smT, 0.0)
                            eb = esb.tile([128, HID], fp32, name="eb", tag="eb")
                            nc.scalar.activation(eb, ea, Act.Exp)
                            ec = esb.tile([128, HID], fp32, name="ec", tag="ec")
                            nc.vector.tensor_scalar(
                                out=ec, in0=smT, scalar1=0.0, scalar2=-1.0,
                                op0=Alu.max, op1=Alu.add)
                            nc.vector.tensor_add(
                                out_tile[:, h * HID:(h + 1) * HID], eb, ec)
                        nc.sync.dma_start(
                            out=out_d[g * 128:(g + 1) * 128, :], in_=out_tile)
    nc.compile()
    return nc


def _get_nc():
    if "nc" not in _cache:
        _cache["nc"] = build()
    return _cache["nc"]


def kernel(x, adj, W, a):
    import sys
    for p in ("/opt/trn_rl_repo", "/opt/trn_rl_repo/concourse"):
        if p not in sys.path:
            sys.path.insert(0, p)
    from concourse.bass_utils import run_bass_kernel_spmd

    x = np.ascontiguousarray(np.asarray(x, dtype=np.float32))
    adj = np.ascontiguousarray(np.asarray(adj, dtype=np.int32))
    W = np.ascontiguousarray(np.asarray(W, dtype=np.float32))
    a = np.ascontiguousarray(np.asarray(a, dtype=np.float32))

    nc = _get_nc()
    in_maps = [
        {
            "x": x,
            "x_blk": np.ascontiguousarray(x[c * NB:(c + 1) * NB]),
            "adj_blk": np.ascontiguousarray(adj[c * NB:(c + 1) * NB]),
            "W": W,
            "a": a,
        }
        for c in range(NCORES)
    ]
    res = run_bass_kernel_spmd(nc, in_maps, core_ids=list(range(NCORES)))
    _cache["last_results"] = res
    out = np.concatenate([r["out_blk"] for r in res.results], axis=0)
    return out.astype(np.float32)



# revision 3
# speedup vs baseline: 1.5044x; 1.5044x over previous
# GAT (graph attention) Trainium2 kernel — 8-core row-parallel SPMD.
#
# Math (per head h, rows I owned by a core):
#   h = x @ W_h ; f1 = h@a1 ; f2 = h@a2 ; z_ij = f1_i + f2_j
#   P_ij = adj_ij ? exp(lrelu(z)) : exp(9e-15 ~= 0) ; att = softmax_j(P)
#   out = elu( (P @ h) / (P @ 1) )
# Device factorization (avoids O(N^2) transcendentals):
#   exp(lrelu(z)) = u'_i * v'_j * max(r_i * w_j, 1)
#     r = e^{0.8 f1}, w = e^{0.8 f2}, u' = e^{0.2 f1}, v' = e^{0.2 f2}
#   E2[j,i] = m^T[j,i] * max(r_i * (w_j v'_j), v'_j)     (ts_dual + tt mult)
#   numer[d,i] = u'_i * ([h|1]^T E2)[d,i] + S[d] - (h^T m^T)[d,i]
#   denom[i]   = u'_i * Y1[i] + N - deg_i
# Transfer-optimized I/O (axon tunnel is ~50 MB/s — e2e is transfer-bound):
#   adj ships bit-packed (np.packbits, 32x smaller); device unpacks via
#   SWDGE u8->bf16 cast load + 8-step is_ge bit-peel (exact in bf16) into
#   the bf16 mask DRAM scratch (with free deg row-sums), then xbar
#   DMA-transpose loads [j-part, i-free] tiles as before.
#   x ships as bf16; out returns as bf16 and is upcast on host.

import numpy as np
import ml_dtypes

N = 8192
EMB = 128
HID = 64
NH = 4
NCORES = 8
NB = N // NCORES  # 1024 rows per core

_cache = {}


def build(n=N, nb=NB):
    import concourse.bass as bass
    import concourse.bacc as bacc
    import concourse.tile as tile
    import concourse.mybir as mybir
    from concourse.masks import make_identity

    fp32 = mybir.dt.float32
    bf16 = mybir.dt.bfloat16
    u8 = mybir.dt.uint8
    Alu = mybir.AluOpType
    Act = mybir.ActivationFunctionType
    MS = bass.MemorySpace

    nbh = nb // 2           # i-half size
    njc = n // 128          # j chunks
    nic = nb // 128         # i chunks (local rows)
    nsub = nbh // 128       # i subchunks per half
    nxc = n // 128          # x chunks

    nc = bacc.Bacc()
    x_d = nc.declare_dram_parameter("x", [n, EMB], bf16, isOutput=False)
    xb_d = nc.declare_dram_parameter("x_blk", [nb, EMB], bf16, isOutput=False)
    adjp_d = nc.declare_dram_parameter("adjp_blk", [nb, n // 8], u8,
                                       isOutput=False)
    W_d = nc.declare_dram_parameter("W", [NH, EMB, HID], fp32, isOutput=False)
    a_d = nc.declare_dram_parameter("a", [NH, 2 * HID, 1], fp32, isOutput=False)
    out_d = nc.declare_dram_parameter("out_blk", [nb, NH * HID], bf16,
                                      isOutput=True)

    with tile.TileContext(nc) as tc:
        with (
            tc.tile_pool(name="const", bufs=1) as const,
            tc.tile_pool(name="ld", bufs=3) as ld,
            tc.tile_pool(name="anat", bufs=2) as anat,
            tc.tile_pool(name="pkp", bufs=1) as pkp,
            tc.tile_pool(name="dramp", bufs=1, space=MS.DRAM) as dramp,
            tc.tile_pool(name="mtp", bufs=6) as mtp,
            tc.tile_pool(name="dep", bufs=8) as dep,
            tc.tile_pool(name="esb", bufs=3) as esb,
        ):
            # ---------------- constants ----------------
            ident = const.tile([128, 128], fp32, name="ident", tag="ident")
            make_identity(nc, ident)
            identb = const.tile([128, 128], bf16, name="identb", tag="identb")
            make_identity(nc, identb)
            ones_row = const.tile([1, 128], fp32, name="ones_row", tag="ones_row")
            nc.vector.memset(ones_row, 1.0)

            madj0 = dramp.tile([nbh, n], bf16, name="madj0", tag="madj0")
            madj1 = dramp.tile([nbh, n], bf16, name="madj1", tag="madj1")
            madj = [madj0, madj1]
            deg_sb = const.tile([128, nic], fp32, name="deg_sb", tag="deg_sb")

            # ------------- stage A: bit-unpack mask (+ deg) -------------
            # adjp rows are packbits(adj_row): byte k bit (7-b) is col 8k+(7-b).
            # Load u8 -> bf16 (values 0..255 exact in bf16), then peel bits
            # MSB-first: bit = (v >= 2^b); v -= 2^b * bit. Strided writes
            # place bit-plane j0=7-b at mask cols j0::8.
            def stageA(ihalf):
                for ics in range(nic // 2):
                    r0 = ihalf * nbh + ics * 128
                    icg = ihalf * (nic // 2) + ics
                    pk = pkp.tile([128, n // 8], bf16, name="pk", tag="pk")
                    nc.gpsimd.dma_start(out=pk, in_=adjp_d[r0:r0 + 128, :])
                    mnat = anat.tile([128, n], bf16, name="mnat", tag="mnat")
                    mv = mnat.rearrange("p (k e) -> p e k", e=8)
                    t0 = pkp.tile([128, n // 8], bf16, name="pt0", tag="pt0")
                    cur, nxt = pk, t0
                    for b in range(7, -1, -1):
                        nc.vector.tensor_scalar(
                            out=mv[:, 7 - b, :], in0=cur,
                            scalar1=float(2 ** b), scalar2=None, op0=Alu.is_ge)
                        if b > 0:
                            nc.vector.scalar_tensor_tensor(
                                out=nxt, in0=mv[:, 7 - b, :],
                                scalar=-float(2 ** b), in1=cur,
                                op0=Alu.mult, op1=Alu.add)
                            cur, nxt = nxt, cur
                    nc.vector.tensor_scalar(
                        out=mnat, in0=mnat, scalar1=1.0, scalar2=None,
                        op0=Alu.mult, op1=Alu.add,
                        accum_out=deg_sb[:, icg:icg + 1])
                    nc.sync.dma_start(
                        out=madj[ihalf][ics * 128:(ics + 1) * 128, :], in_=mnat)

            stageA(0)

            # ---------------- prologue ----------------
            ppsum = tc.alloc_tile_pool(name="ppsum", bufs=2, space=MS.PSUM)
            Wsb = const.tile([128, NH, HID], fp32, name="Wsb", tag="Wsb")
            nc.sync.dma_start(out=Wsb, in_=W_d.rearrange("h e d -> e h d"))
            asb = const.tile([HID, NH, 2], fp32, name="asb", tag="asb")
            nc.sync.dma_start(
                out=asb, in_=a_d.rearrange("h (t d) o -> d h (t o)", t=2))

            # x^T  [128e, n]  (x arrives bf16; transpose via bf16 identity,
            # accumulate/copy to fp32)
            xT = const.tile([128, n], fp32, name="xT", tag="xT")
            for ic in range(nxc):
                xt_nat = ld.tile([128, EMB], bf16, name="xt_nat", tag="xt_nat")
                nc.sync.dma_start(out=xt_nat, in_=x_d[ic * 128:(ic + 1) * 128, :])
                ps = ppsum.tile([128, 128], fp32, name="ps", tag="ps")
                nc.tensor.matmul(ps, xt_nat, identb)
                nc.scalar.copy(out=xT[:, ic * 128:(ic + 1) * 128], in_=ps)
            # x_blk^T [128e, nb]
            xbT = const.tile([128, nb], fp32, name="xbT", tag="xbT")
            for ic in range(nic):
                xb_nat = ld.tile([128, EMB], bf16, name="xb_nat", tag="xt_nat")
                nc.sync.dma_start(out=xb_nat, in_=xb_d[ic * 128:(ic + 1) * 128, :])
                ps = ppsum.tile([128, 128], fp32, name="ps", tag="ps")
                nc.tensor.matmul(ps, xb_nat, identb)
                nc.scalar.copy(out=xbT[:, ic * 128:(ic + 1) * 128], in_=ps)

            # xsum[e] = sum_j x[j,e]
            xsum = const.tile([128, 1], fp32, name="xsum", tag="xsum")
            nc.vector.tensor_reduce(xsum, xT, mybir.AxisListType.X, Alu.add)

            # W^T per head; q = [W a1 | W a2] -> Qsb [128e, NH, 2]
            WTsb = const.tile([HID, NH, 128], fp32, name="WTsb", tag="WTsb")
            Qsb = const.tile([128, NH, 2], fp32, name="Qsb", tag="Qsb")
            for h in range(NH):
                wt_ps = ppsum.tile([HID, 128], fp32, name="wt_ps", tag="ps")
                nc.tensor.matmul(wt_ps, Wsb[:, h, :], ident)
                nc.scalar.copy(out=WTsb[:, h, :], in_=wt_ps)
                q_ps = ppsum.tile([128, 2], fp32, name="q_ps", tag="ps")
                nc.tensor.matmul(q_ps, WTsb[:, h, :], asb[:, h, :])
                nc.scalar.copy(out=Qsb[:, h, :], in_=q_ps)

            Qflat = Qsb.rearrange("p h t -> p (h t)")
            Wflat = Wsb.rearrange("e h d -> e (h d)")

            # f columns for all j: Fcol[p, jc, (h t)] = f_{t,h}[jc*128+p]
            Fcol = const.tile([128, njc, 2 * NH], fp32, name="Fcol", tag="Fcol")
            for jc in range(njc):
                f_ps = ppsum.tile([128, 2 * NH], fp32, name="f_ps", tag="ps")
                nc.tensor.matmul(f_ps, xT[:, jc * 128:(jc + 1) * 128], Qflat)
                nc.scalar.copy(out=Fcol[:, jc, :], in_=f_ps)

            # f rows for local block: Frow [8, nb]
            Frow = const.tile([2 * NH, nb], fp32, name="Frow", tag="Frow")
            for half in range(2):
                fr_ps = ppsum.tile([2 * NH, nbh], fp32, name="fr_ps", tag="ps")
                nc.tensor.matmul(fr_ps, Qflat, xbT[:, half * nbh:(half + 1) * nbh])
                nc.scalar.copy(out=Frow[:, half * nbh:(half + 1) * nbh], in_=fr_ps)

            # FrowT [128, nic, 8]
            FrowT = const.tile([128, nic, 2 * NH], fp32, name="FrowT", tag="FrowT")
            for g in range(nic):
                ft_ps = ppsum.tile([128, 2 * NH], fp32, name="ft_ps", tag="ps")
                nc.tensor.matmul(
                    ft_ps, Frow[:, g * 128:(g + 1) * 128],
                    ident[0:2 * NH, 0:2 * NH])
                nc.scalar.copy(out=FrowT[:, g, :], in_=ft_ps)

            # scalar cols (j side): ETc = e^{f2} (= w v'), Vc = e^{0.2 f2}
            ETc = const.tile([128, njc, NH], fp32, name="ETc", tag="ETc")
            Vc = const.tile([128, njc, NH], fp32, name="Vc", tag="Vc")
            for h in range(NH):
                nc.scalar.activation(ETc[:, :, h], Fcol[:, :, 2 * h + 1], Act.Exp)
                nc.scalar.activation(
                    Vc[:, :, h], Fcol[:, :, 2 * h + 1], Act.Exp, scale=0.2)

            # row side: R8 = e^{0.8 Frow}; U'T = e^{0.2 FrowT}
            R8 = const.tile([2 * NH, nb], fp32, name="R8", tag="R8")
            nc.scalar.activation(R8, Frow, Act.Exp, scale=0.8)
            UpT = const.tile([128, nic, 2 * NH], fp32, name="UpT", tag="UpT")
            nc.scalar.activation(
                UpT.rearrange("p a b -> p (a b)"),
                FrowT.rearrange("p a b -> p (a b)"), Act.Exp, scale=0.2)

            # r broadcast per head [128, nb] bf16: bounce rows via DRAM, then
            # broadcast-load with stride-0 partition AP (+ cast) via SWDGE.
            r8_dram = dramp.tile([2 * NH, nb], fp32, name="r8_dram", tag="r8d")
            nc.sync.dma_start(out=r8_dram, in_=R8)
            rbc = []
            for h in range(NH):
                t = const.tile([128, nb], bf16, name=f"rbc{h}", tag=f"rbc{h}")
                srow = r8_dram[2 * h:2 * h + 1, :]
                src_b = bass.AP(
                    tensor=srow.tensor, offset=srow.offset,
                    ap=[[0, 128]] + [list(d) for d in srow.ap[1:]])
                nc.gpsimd.dma_start(out=t, in_=src_b)
                rbc.append(t)

            # H~ [128, njc, NH, HID+1] bf16 (ones col at [.., HID]) for the
            # per-head X passes, plus a contiguous pair layout for hm passes
            # (matmul weights APs must have a single free dimension).
            Hsb = const.tile([128, njc, NH, HID + 1], bf16, name="Hsb", tag="Hsb")
            Hpair = const.tile([128, njc, NH * HID], bf16, name="Hpair", tag="Hpair")
            nc.vector.memset(Hsb[:, :, :, HID], 1.0)
            for jc in range(njc):
                h_ps = ppsum.tile([128, NH, HID], fp32, name="h_ps", tag="ps")
                nc.tensor.matmul(
                    h_ps.rearrange("p h d -> p (h d)"),
                    xT[:, jc * 128:(jc + 1) * 128], Wflat)
                nc.scalar.copy(out=Hsb[:, jc, :, 0:HID], in_=h_ps)
                nc.scalar.copy(
                    out=Hpair[:, jc, :].rearrange("p (h d) -> p h d", h=NH),
                    in_=h_ps)

            # S row then per-head broadcast [128, HID]
            s_ps = ppsum.tile([1, NH * HID], fp32, name="s_ps", tag="ps")
            nc.tensor.matmul(s_ps, xsum, Wflat)
            S_row = const.tile([1, NH * HID], fp32, name="S_row", tag="S_row")
            nc.scalar.copy(out=S_row, in_=s_ps)
            Sb = []
            for h in range(NH):
                sb_ps = ppsum.tile([128, HID], fp32, name="sb_ps", tag="ps")
                nc.tensor.matmul(sb_ps, ones_row, S_row[:, h * HID:(h + 1) * HID])
                t = const.tile([128, HID], fp32, name=f"Sb{h}", tag=f"Sb{h}")
                nc.scalar.copy(out=t, in_=sb_ps)
                Sb.append(t)

            ppsum.release()

            # ------------- stage A part 2, then degbar -------------
            stageA(1)
            degbar = const.tile([128, nic], fp32, name="degbar", tag="degbar")
            nc.vector.tensor_scalar(
                out=degbar, in0=deg_sb, scalar1=-1.0, scalar2=float(n),
                op0=Alu.mult, op1=Alu.add)

            # ---------------- main loop ----------------
            for ihalf in range(2):
                with (
                    tc.tile_pool(name=f"mm{ihalf}", bufs=1, space=MS.PSUM) as mm,
                    tc.tile_pool(name=f"ep{ihalf}", bufs=2, space=MS.PSUM) as ep,
                ):
                    X = [mm.tile([HID + 1, nbh], fp32, name=f"X{h}", tag=f"X{h}")
                         for h in range(NH)]
                    HM = [mm.tile([128, nbh], fp32, name=f"HM{p}", tag=f"HM{p}")
                          for p in range(2)]
                    for jc in range(njc):
                        mT = mtp.tile([128, nbh], bf16, name="mT", tag="mT")
                        nc.sync.dma_start_transpose(
                            out=mT,
                            in_=madj[ihalf][:, jc * 128:(jc + 1) * 128])
                        for h in range(NH):
                            D2 = dep.tile([128, nbh], bf16, name="D2", tag="D2")
                            nc.vector.tensor_scalar(
                                out=D2,
                                in0=rbc[h][:, ihalf * nbh:(ihalf + 1) * nbh],
                                scalar1=ETc[:, jc, h:h + 1],
                                scalar2=Vc[:, jc, h:h + 1],
                                op0=Alu.mult, op1=Alu.max)
                            E2 = dep.tile([128, nbh], bf16, name="E2", tag="E2")
                            eng_tt = nc.gpsimd if h >= 2 else nc.vector
                            eng_tt.tensor_mul(E2, mT, D2)
                            nc.tensor.matmul(
                                X[h], Hsb[:, jc, h, :], E2,
                                start=(jc == 0), stop=(jc == njc - 1))
                        for p in range(2):
                            nc.tensor.matmul(
                                HM[p],
                                Hpair[:, jc, 128 * p:128 * (p + 1)], mT,
                                start=(jc == 0), stop=(jc == njc - 1))

                    # ---------------- epilogue for this half ----------------
                    XS = []
                    for h in range(NH):
                        t = esb.tile([HID + 1, nbh], fp32,
                                     name=f"XS{h}", tag=f"XS{h}", bufs=1)
                        nc.scalar.copy(out=t, in_=X[h])
                        XS.append(t)
                    HMS = []
                    for p in range(2):
                        t = esb.tile([128, nbh], fp32,
                                     name=f"HMS{p}", tag=f"HMS{p}", bufs=1)
                        nc.scalar.copy(out=t, in_=HM[p])
                        HMS.append(t)

                    for isub in range(nsub):
                        g = ihalf * nsub + isub
                        sl = slice(isub * 128, (isub + 1) * 128)
                        hmT = []
                        for p in range(2):
                            tp = ep.tile([128, 128], fp32, name="tp", tag="tp")
                            nc.tensor.matmul(tp, HMS[p][:, sl], ident)
                            t = esb.tile([128, 128], fp32,
                                         name=f"hmT{p}", tag=f"hmT{p}", bufs=2)
                            nc.scalar.copy(out=t, in_=tp)
                            hmT.append(t)
                        out_tile = esb.tile([128, NH * HID], bf16,
                                            name="out_tile", tag="otile", bufs=2)
                        for h in range(NH):
                            tp = ep.tile([128, HID + 1], fp32, name="tpx", tag="tp")
                            nc.tensor.matmul(
                                tp, XS[h][:, sl], ident[0:HID + 1, 0:HID + 1])
                            XT = esb.tile([128, HID + 1], fp32, name="XT", tag="XT")
                            nc.scalar.copy(out=XT, in_=tp)
                            upc = UpT[:, g, 2 * h:2 * h + 1]
                            n1 = esb.tile([128, HID], fp32, name="n1", tag="n1")
                            nc.vector.tensor_scalar(
                                out=n1, in0=XT[:, 0:HID], scalar1=upc,
                                scalar2=None, op0=Alu.mult)
                            n2 = esb.tile([128, HID], fp32, name="n2", tag="n2")
                            nc.vector.scalar_tensor_tensor(
                                out=n2,
                                in0=hmT[h // 2][:, (h % 2) * HID:
                                                (h % 2) * HID + HID],
                                scalar=-1.0, in1=n1, op0=Alu.mult, op1=Alu.add)
                            n3 = esb.tile([128, HID], fp32, name="n3", tag="n3")
                            nc.vector.tensor_add(n3, n2, Sb[h])
                            dcol = esb.tile([128, 1], fp32, name="dcol", tag="dcol")
                            nc.vector.tensor_scalar(
                                out=dcol, in0=XT[:, HID:HID + 1], scalar1=upc,
                                scalar2=degbar[:, g:g + 1],
                                op0=Alu.mult, op1=Alu.add)
                            rec = esb.tile([128, 1], fp32, name="rec", tag="rec")
                            nc.vector.reciprocal(rec, dcol)
                            smT = esb.tile([128, HID], fp32, name="smT", tag="smT")
                            nc.vector.tensor_scalar(
                                out=smT, in0=n3, scalar1=rec, scalar2=None,
                                op0=Alu.mult)
                            # elu = (max(sm,0)-1) + exp(min(sm,0))
                            ea = esb.tile([128, HID], fp32, name="ea", tag="ea")
                            nc.vector.tensor_scalar_min(ea, smT, 0.0)
                            eb = esb.tile([128, HID], fp32, name="eb", tag="eb")
                            nc.scalar.activation(eb, ea, Act.Exp)
                            ec = esb.tile([128, HID], fp32, name="ec", tag="ec")
                            nc.vector.tensor_scalar(
                                out=ec, in0=smT, scalar1=0.0, scalar2=-1.0,
                                op0=Alu.max, op1=Alu.add)
                            nc.vector.tensor_add(
                                out_tile[:, h * HID:(h + 1) * HID], eb, ec)
                        nc.sync.dma_start(
                            out=out_d[g * 128:(g + 1) * 128, :], in_=out_tile)
    nc.compile()
    return nc


def _get_nc():
    if "nc" not in _cache:
        _cache["nc"] = build()
    return _cache["nc"]


def kernel(x, adj, W, a):
    import sys
    for p in ("/opt/trn_rl_repo", "/opt/trn_rl_repo/concourse"):
        if p not in sys.path:
            sys.path.insert(0, p)
    from concourse.bass_utils import run_bass_kernel_spmd

    x = np.asarray(x)
    adj = np.asarray(adj)
    W = np.ascontiguousarray(np.asarray(W, dtype=np.float32))
    a = np.ascontiguousarray(np.asarray(a, dtype=np.float32))

    xb = np.ascontiguousarray(x).astype(ml_dtypes.bfloat16)
    if adj.dtype == np.int32 and adj.flags.c_contiguous:
        # little-endian low byte of each 0/1 int32 is the value itself
        adjp = np.packbits(adj.view(np.uint8)[:, ::4], axis=1)
    else:
        adjp = np.packbits(adj.astype(np.uint8), axis=1)

    nc = _get_nc()
    in_maps = [
        {
            "x": xb,
            "x_blk": xb[c * NB:(c + 1) * NB],
            "adjp_blk": adjp[c * NB:(c + 1) * NB],
            "W": W,
            "a": a,
        }
        for c in range(NCORES)
    ]
    res = run_bass_kernel_spmd(nc, in_maps, core_ids=list(range(NCORES)))
    _cache["last_results"] = res
    out = np.concatenate([r["out_blk"] for r in res.results], axis=0)
    return out.astype(np.float32)


# revision 4
# speedup vs baseline: 4.8832x; 3.2459x over previous
# GAT (graph attention) Trainium2 kernel — 8-core row-parallel SPMD.
#
# Math (per head h, rows I owned by a core):
#   h = x @ W_h ; f1 = h@a1 ; f2 = h@a2 ; z_ij = f1_i + f2_j
#   P_ij = adj_ij ? exp(lrelu(z)) : exp(9e-15 ~= 0) ; att = softmax_j(P)
#   out = elu( (P @ h) / (P @ 1) )
# Device factorization (avoids O(N^2) transcendentals):
#   exp(lrelu(z)) = u'_i * v'_j * max(r_i * w_j, 1)
#     r = e^{0.8 f1}, w = e^{0.8 f2}, u' = e^{0.2 f1}, v' = e^{0.2 f2}
#   E2[j,i] = m^T[j,i] * max(r_i * (w_j v'_j), v'_j)     (ts_dual + tt mult)
#   numer[d,i] = u'_i * ([h|1]^T E2)[d,i] + S[d] - (h^T m^T)[d,i]
#   denom[i]   = u'_i * Y1[i] + N - deg_i
# Transfer-optimized I/O (axon tunnel is ~50 MB/s — e2e is transfer-bound):
#   adj ships bit-packed (np.packbits, 32x smaller); device unpacks via
#   SWDGE u8->bf16 cast load + 8-step is_ge bit-peel (exact in bf16) into
#   the bf16 mask DRAM scratch (with free deg row-sums), then xbar
#   DMA-transpose loads [j-part, i-free] tiles as before.
#   x ships as bf16; out returns as bf16 and is upcast on host.

import numpy as np
import ml_dtypes

N = 8192
EMB = 128
HID = 64
NH = 4
NCORES = 8
NB = N // NCORES  # 1024 rows per core

_cache = {}


def build(n=N, nb=NB):
    import concourse.bass as bass
    import concourse.bacc as bacc
    import concourse.tile as tile
    import concourse.mybir as mybir
    from concourse.masks import make_identity

    fp32 = mybir.dt.float32
    bf16 = mybir.dt.bfloat16
    u8 = mybir.dt.uint8
    Alu = mybir.AluOpType
    Act = mybir.ActivationFunctionType
    MS = bass.MemorySpace

    nbh = nb // 2           # i-half size
    njc = n // 128          # j chunks
    nic = nb // 128         # i chunks (local rows)
    nsub = nbh // 128       # i subchunks per half
    nxc = n // 128          # x chunks

    nc = bacc.Bacc()
    x_d = nc.declare_dram_parameter("x", [n, EMB], bf16, isOutput=False)
    xb_d = nc.declare_dram_parameter("x_blk", [nb, EMB], bf16, isOutput=False)
    adjp_d = nc.declare_dram_parameter("adjp_blk", [nb, n // 8], u8,
                                       isOutput=False)
    W_d = nc.declare_dram_parameter("W", [NH, EMB, HID], fp32, isOutput=False)
    a_d = nc.declare_dram_parameter("a", [NH, 2 * HID, 1], fp32, isOutput=False)
    out_d = nc.declare_dram_parameter("out_blk", [nb, NH * HID], bf16,
                                      isOutput=True)

    with tile.TileContext(nc) as tc:
        with (
            tc.tile_pool(name="const", bufs=1) as const,
            tc.tile_pool(name="ld", bufs=3) as ld,
            tc.tile_pool(name="anat", bufs=2) as anat,
            tc.tile_pool(name="pkp", bufs=1) as pkp,
            tc.tile_pool(name="dramp", bufs=1, space=MS.DRAM) as dramp,
            tc.tile_pool(name="mtp", bufs=6) as mtp,
            tc.tile_pool(name="dep", bufs=8) as dep,
            tc.tile_pool(name="esb", bufs=3) as esb,
        ):
            # ---------------- constants ----------------
            ident = const.tile([128, 128], fp32, name="ident", tag="ident")
            make_identity(nc, ident)
            identb = const.tile([128, 128], bf16, name="identb", tag="identb")
            make_identity(nc, identb)
            ones_row = const.tile([1, 128], fp32, name="ones_row", tag="ones_row")
            nc.vector.memset(ones_row, 1.0)

            madj0 = dramp.tile([nbh, n], bf16, name="madj0", tag="madj0")
            madj1 = dramp.tile([nbh, n], bf16, name="madj1", tag="madj1")
            madj = [madj0, madj1]
            deg_sb = const.tile([128, nic], fp32, name="deg_sb", tag="deg_sb")

            # ------------- stage A: bit-unpack mask (+ deg) -------------
            # adjp rows are packbits(adj_row): byte k bit (7-b) is col 8k+(7-b).
            # Load u8 -> bf16 (values 0..255 exact in bf16), then peel bits
            # MSB-first: bit = (v >= 2^b); v -= 2^b * bit. Strided writes
            # place bit-plane j0=7-b at mask cols j0::8.
            def stageA(ihalf):
                for ics in range(nic // 2):
                    r0 = ihalf * nbh + ics * 128
                    icg = ihalf * (nic // 2) + ics
                    pk = pkp.tile([128, n // 8], bf16, name="pk", tag="pk")
                    nc.gpsimd.dma_start(out=pk, in_=adjp_d[r0:r0 + 128, :])
                    mnat = anat.tile([128, n], bf16, name="mnat", tag="mnat")
                    mv = mnat.rearrange("p (k e) -> p e k", e=8)
                    t0 = pkp.tile([128, n // 8], bf16, name="pt0", tag="pt0")
                    cur, nxt = pk, t0
                    for b in range(7, -1, -1):
                        nc.vector.tensor_scalar(
                            out=mv[:, 7 - b, :], in0=cur,
                            scalar1=float(2 ** b), scalar2=None, op0=Alu.is_ge)
                        if b > 0:
                            nc.vector.scalar_tensor_tensor(
                                out=nxt, in0=mv[:, 7 - b, :],
                                scalar=-float(2 ** b), in1=cur,
                                op0=Alu.mult, op1=Alu.add)
                            cur, nxt = nxt, cur
                    nc.vector.tensor_scalar(
                        out=mnat, in0=mnat, scalar1=1.0, scalar2=None,
                        op0=Alu.mult, op1=Alu.add,
                        accum_out=deg_sb[:, icg:icg + 1])
                    nc.sync.dma_start(
                        out=madj[ihalf][ics * 128:(ics + 1) * 128, :], in_=mnat)

            stageA(0)

            # ---------------- prologue ----------------
            ppsum = tc.alloc_tile_pool(name="ppsum", bufs=2, space=MS.PSUM)
            Wsb = const.tile([128, NH, HID], fp32, name="Wsb", tag="Wsb")
            nc.sync.dma_start(out=Wsb, in_=W_d.rearrange("h e d -> e h d"))
            asb = const.tile([HID, NH, 2], fp32, name="asb", tag="asb")
            nc.sync.dma_start(
                out=asb, in_=a_d.rearrange("h (t d) o -> d h (t o)", t=2))

            # x^T  [128e, n]  (x arrives bf16; transpose via bf16 identity,
            # accumulate/copy to fp32)
            xT = const.tile([128, n], fp32, name="xT", tag="xT")
            for ic in range(nxc):
                xt_nat = ld.tile([128, EMB], bf16, name="xt_nat", tag="xt_nat")
                nc.sync.dma_start(out=xt_nat, in_=x_d[ic * 128:(ic + 1) * 128, :])
                ps = ppsum.tile([128, 128], fp32, name="ps", tag="ps")
                nc.tensor.matmul(ps, xt_nat, identb)
                nc.scalar.copy(out=xT[:, ic * 128:(ic + 1) * 128], in_=ps)
            # x_blk^T [128e, nb]
            xbT = const.tile([128, nb], fp32, name="xbT", tag="xbT")
            for ic in range(nic):
                xb_nat = ld.tile([128, EMB], bf16, name="xb_nat", tag="xt_nat")
                nc.sync.dma_start(out=xb_nat, in_=xb_d[ic * 128:(ic + 1) * 128, :])
                ps = ppsum.tile([128, 128], fp32, name="ps", tag="ps")
                nc.tensor.matmul(ps, xb_nat, identb)
                nc.scalar.copy(out=xbT[:, ic * 128:(ic + 1) * 128], in_=ps)

            # xsum[e] = sum_j x[j,e]
            xsum = const.tile([128, 1], fp32, name="xsum", tag="xsum")
            nc.vector.tensor_reduce(xsum, xT, mybir.AxisListType.X, Alu.add)

            # W^T per head; q = [W a1 | W a2] -> Qsb [128e, NH, 2]
            WTsb = const.tile([HID, NH, 128], fp32, name="WTsb", tag="WTsb")
            Qsb = const.tile([128, NH, 2], fp32, name="Qsb", tag="Qsb")
            for h in range(NH):
                wt_ps = ppsum.tile([HID, 128], fp32, name="wt_ps", tag="ps")
                nc.tensor.matmul(wt_ps, Wsb[:, h, :], ident)
                nc.scalar.copy(out=WTsb[:, h, :], in_=wt_ps)
                q_ps = ppsum.tile([128, 2], fp32, name="q_ps", tag="ps")
                nc.tensor.matmul(q_ps, WTsb[:, h, :], asb[:, h, :])
                nc.scalar.copy(out=Qsb[:, h, :], in_=q_ps)

            Qflat = Qsb.rearrange("p h t -> p (h t)")
            Wflat = Wsb.rearrange("e h d -> e (h d)")

            # f columns for all j: Fcol[p, jc, (h t)] = f_{t,h}[jc*128+p]
            Fcol = const.tile([128, njc, 2 * NH], fp32, name="Fcol", tag="Fcol")
            for jc in range(njc):
                f_ps = ppsum.tile([128, 2 * NH], fp32, name="f_ps", tag="ps")
                nc.tensor.matmul(f_ps, xT[:, jc * 128:(jc + 1) * 128], Qflat)
                nc.scalar.copy(out=Fcol[:, jc, :], in_=f_ps)

            # f rows for local block: Frow [8, nb]
            Frow = const.tile([2 * NH, nb], fp32, name="Frow", tag="Frow")
            for half in range(2):
                fr_ps = ppsum.tile([2 * NH, nbh], fp32, name="fr_ps", tag="ps")
                nc.tensor.matmul(fr_ps, Qflat, xbT[:, half * nbh:(half + 1) * nbh])
                nc.scalar.copy(out=Frow[:, half * nbh:(half + 1) * nbh], in_=fr_ps)

            # FrowT [128, nic, 8]
            FrowT = const.tile([128, nic, 2 * NH], fp32, name="FrowT", tag="FrowT")
            for g in range(nic):
                ft_ps = ppsum.tile([128, 2 * NH], fp32, name="ft_ps", tag="ps")
                nc.tensor.matmul(
                    ft_ps, Frow[:, g * 128:(g + 1) * 128],
                    ident[0:2 * NH, 0:2 * NH])
                nc.scalar.copy(out=FrowT[:, g, :], in_=ft_ps)

            # scalar cols (j side): ETc = e^{f2} (= w v'), Vc = e^{0.2 f2}
            ETc = const.tile([128, njc, NH], fp32, name="ETc", tag="ETc")
            Vc = const.tile([128, njc, NH], fp32, name="Vc", tag="Vc")
            for h in range(NH):
                nc.scalar.activation(ETc[:, :, h], Fcol[:, :, 2 * h + 1], Act.Exp)
                nc.scalar.activation(
                    Vc[:, :, h], Fcol[:, :, 2 * h + 1], Act.Exp, scale=0.2)

            # row side: R8 = e^{0.8 Frow}; U'T = e^{0.2 FrowT}
            R8 = const.tile([2 * NH, nb], fp32, name="R8", tag="R8")
            nc.scalar.activation(R8, Frow, Act.Exp, scale=0.8)
            UpT = const.tile([128, nic, 2 * NH], fp32, name="UpT", tag="UpT")
            nc.scalar.activation(
                UpT.rearrange("p a b -> p (a b)"),
                FrowT.rearrange("p a b -> p (a b)"), Act.Exp, scale=0.2)

            # r broadcast per head [128, nb] bf16: bounce rows via DRAM, then
            # broadcast-load with stride-0 partition AP (+ cast) via SWDGE.
            r8_dram = dramp.tile([2 * NH, nb], fp32, name="r8_dram", tag="r8d")
            nc.sync.dma_start(out=r8_dram, in_=R8)
            rbc = []
            for h in range(NH):
                t = const.tile([128, nb], bf16, name=f"rbc{h}", tag=f"rbc{h}")
                srow = r8_dram[2 * h:2 * h + 1, :]
                src_b = bass.AP(
                    tensor=srow.tensor, offset=srow.offset,
                    ap=[[0, 128]] + [list(d) for d in srow.ap[1:]])
                nc.gpsimd.dma_start(out=t, in_=src_b)
                rbc.append(t)

            # H~ [128, njc, NH, HID+1] bf16 (ones col at [.., HID]) for the
            # per-head X passes, plus a contiguous pair layout for hm passes
            # (matmul weights APs must have a single free dimension).
            Hsb = const.tile([128, njc, NH, HID + 1], bf16, name="Hsb", tag="Hsb")
            Hpair = const.tile([128, njc, NH * HID], bf16, name="Hpair", tag="Hpair")
            nc.vector.memset(Hsb[:, :, :, HID], 1.0)
            for jc in range(njc):
                h_ps = ppsum.tile([128, NH, HID], fp32, name="h_ps", tag="ps")
                nc.tensor.matmul(
                    h_ps.rearrange("p h d -> p (h d)"),
                    xT[:, jc * 128:(jc + 1) * 128], Wflat)
                nc.scalar.copy(out=Hsb[:, jc, :, 0:HID], in_=h_ps)
                nc.scalar.copy(
                    out=Hpair[:, jc, :].rearrange("p (h d) -> p h d", h=NH),
                    in_=h_ps)

            # S row then per-head broadcast [128, HID]
            s_ps = ppsum.tile([1, NH * HID], fp32, name="s_ps", tag="ps")
            nc.tensor.matmul(s_ps, xsum, Wflat)
            S_row = const.tile([1, NH * HID], fp32, name="S_row", tag="S_row")
            nc.scalar.copy(out=S_row, in_=s_ps)
            Sb = []
            for h in range(NH):
                sb_ps = ppsum.tile([128, HID], fp32, name="sb_ps", tag="ps")
                nc.tensor.matmul(sb_ps, ones_row, S_row[:, h * HID:(h + 1) * HID])
                t = const.tile([128, HID], fp32, name=f"Sb{h}", tag=f"Sb{h}")
                nc.scalar.copy(out=t, in_=sb_ps)
                Sb.append(t)

            ppsum.release()

            # ------------- stage A part 2, then degbar -------------
            stageA(1)
            degbar = const.tile([128, nic], fp32, name="degbar", tag="degbar")
            nc.vector.tensor_scalar(
                out=degbar, in0=deg_sb, scalar1=-1.0, scalar2=float(n),
                op0=Alu.mult, op1=Alu.add)

            # ---------------- main loop ----------------
            for ihalf in range(2):
                with (
                    tc.tile_pool(name=f"mm{ihalf}", bufs=1, space=MS.PSUM) as mm,
                    tc.tile_pool(name=f"ep{ihalf}", bufs=2, space=MS.PSUM) as ep,
                ):
                    X = [mm.tile([HID + 1, nbh], fp32, name=f"X{h}", tag=f"X{h}")
                         for h in range(NH)]
                    HM = [mm.tile([128, nbh], fp32, name=f"HM{p}", tag=f"HM{p}")
                          for p in range(2)]
                    for jc in range(njc):
                        mT = mtp.tile([128, nbh], bf16, name="mT", tag="mT")
                        nc.sync.dma_start_transpose(
                            out=mT,
                            in_=madj[ihalf][:, jc * 128:(jc + 1) * 128])
                        for h in range(NH):
                            D2 = dep.tile([128, nbh], bf16, name="D2", tag="D2")
                            nc.vector.tensor_scalar(
                                out=D2,
                                in0=rbc[h][:, ihalf * nbh:(ihalf + 1) * nbh],
                                scalar1=ETc[:, jc, h:h + 1],
                                scalar2=Vc[:, jc, h:h + 1],
                                op0=Alu.mult, op1=Alu.max)
                            E2 = dep.tile([128, nbh], bf16, name="E2", tag="E2")
                            eng_tt = nc.gpsimd if h >= 2 else nc.vector
                            eng_tt.tensor_mul(E2, mT, D2)
                            nc.tensor.matmul(
                                X[h], Hsb[:, jc, h, :], E2,
                                start=(jc == 0), stop=(jc == njc - 1))
                        for p in range(2):
                            nc.tensor.matmul(
                                HM[p],
                                Hpair[:, jc, 128 * p:128 * (p + 1)], mT,
                                start=(jc == 0), stop=(jc == njc - 1))

                    # ---------------- epilogue for this half ----------------
                    XS = []
                    for h in range(NH):
                        t = esb.tile([HID + 1, nbh], fp32,
                                     name=f"XS{h}", tag=f"XS{h}", bufs=1)
                        nc.scalar.copy(out=t, in_=X[h])
                        XS.append(t)
                    HMS = []
                    for p in range(2):
                        t = esb.tile([128, nbh], fp32,
                                     name=f"HMS{p}", tag=f"HMS{p}", bufs=1)
                        nc.scalar.copy(out=t, in_=HM[p])
                        HMS.append(t)

                    for isub in range(nsub):
                        g = ihalf * nsub + isub
                        sl = slice(isub * 128, (isub + 1) * 128)
                        hmT = []
                        for p in range(2):
                            tp = ep.tile([128, 128], fp32, name="tp", tag="tp")
                            nc.tensor.matmul(tp, HMS[p][:, sl], ident)
                            t = esb.tile([128, 128], fp32,
                                         name=f"hmT{p}", tag=f"hmT{p}", bufs=2)
                            nc.scalar.copy(out=t, in_=tp)
                            hmT.append(t)
                        out_tile = esb.tile([128, NH * HID], bf16,
                                            name="out_tile", tag="otile", bufs=2)
                        for h in range(NH):
                            tp = ep.tile([128, HID + 1], fp32, name="tpx", tag="tp")
                            nc.tensor.matmul(
                                tp, XS[h][:, sl], ident[0:HID + 1, 0:HID + 1])
                            XT = esb.tile([128, HID + 1], fp32, name="XT", tag="XT")
                            nc.scalar.copy(out=XT, in_=tp)
                            upc = UpT[:, g, 2 * h:2 * h + 1]
                            n1 = esb.tile([128, HID], fp32, name="n1", tag="n1")
                            nc.vector.tensor_scalar(
                                out=n1, in0=XT[:, 0:HID], scalar1=upc,
                                scalar2=None, op0=Alu.mult)
                            n2 = esb.tile([128, HID], fp32, name="n2", tag="n2")
                            nc.vector.scalar_tensor_tensor(
                                out=n2,
                                in0=hmT[h // 2][:, (h % 2) * HID:
                                                (h % 2) * HID + HID],
                                scalar=-1.0, in1=n1, op0=Alu.mult, op1=Alu.add)
                            n3 = esb.tile([128, HID], fp32, name="n3", tag="n3")
                            nc.vector.tensor_add(n3, n2, Sb[h])
                            dcol = esb.tile([128, 1], fp32, name="dcol", tag="dcol")
                            nc.vector.tensor_scalar(
                                out=dcol, in0=XT[:, HID:HID + 1], scalar1=upc,
                                scalar2=degbar[:, g:g + 1],
                                op0=Alu.mult, op1=Alu.add)
                            rec = esb.tile([128, 1], fp32, name="rec", tag="rec")
                            nc.vector.reciprocal(rec, dcol)
                            smT = esb.tile([128, HID], fp32, name="smT", tag="smT")
                            nc.vector.tensor_scalar(
                                out=smT, in0=n3, scalar1=rec, scalar2=None,
                                op0=Alu.mult)
                            # elu = (max(sm,0)-1) + exp(min(sm,0))
                            ea = esb.tile([128, HID], fp32, name="ea", tag="ea")
                            nc.vector.tensor_scalar_min(ea, smT, 0.0)
                            eb = esb.tile([128, HID], fp32, name="eb", tag="eb")
                            nc.scalar.activation(eb, ea, Act.Exp)
                            ec = esb.tile([128, HID], fp32, name="ec", tag="ec")
                            nc.vector.tensor_scalar(
                                out=ec, in0=smT, scalar1=0.0, scalar2=-1.0,
                                op0=Alu.max, op1=Alu.add)
                            nc.vector.tensor_add(
                                out_tile[:, h * HID:(h + 1) * HID], eb, ec)
                        nc.sync.dma_start(
                            out=out_d[g * 128:(g + 1) * 128, :], in_=out_tile)
    nc.compile()
    return nc


def _get_nc():
    if "nc" not in _cache:
        _cache["nc"] = build()
    return _cache["nc"]


def _get_runner():
    # run_bass_via_pjrt rebuilds jax.jit(shard_map(closure)) per call, so the
    # jit cache misses every time (multi-second retrace+compile). Build the
    # jitted sharded callable ONCE and reuse it.
    if "runner" in _cache:
        return _cache["runner"]

    import jax
    from jax.sharding import Mesh, PartitionSpec
    from jax.experimental.shard_map import shard_map
    import concourse.mybir as mybir
    from concourse.bass2jax import (
        _bass_exec_p, partition_id_tensor, install_neuronx_cc_hook)

    nc = _get_nc()
    install_neuronx_cc_hook()
    partition_name = (
        nc.partition_id_tensor.name if nc.partition_id_tensor else None)
    in_names, out_names, out_avals = [], [], []
    for alloc in nc.m.functions[0].allocations:
        if not isinstance(alloc, mybir.MemoryLocationSet):
            continue
        name = alloc.memorylocations[0].name
        if alloc.kind == "ExternalInput":
            if name != partition_name:
                in_names.append(name)
        elif alloc.kind == "ExternalOutput":
            out_names.append(name)
            out_avals.append(jax.core.ShapedArray(
                tuple(alloc.tensor_shape), mybir.dt.np(alloc.dtype)))
    n_params = len(in_names)
    n_outs = len(out_avals)
    all_names = in_names + out_names
    if partition_name is not None:
        all_names = all_names + [partition_name]
    donate = tuple(range(n_params, n_params + n_outs))

    def _body(*args):
        operands = list(args)
        if partition_name is not None:
            operands.append(partition_id_tensor())
        outs = _bass_exec_p.bind(
            *operands, out_avals=tuple(out_avals),
            in_names=tuple(all_names), out_names=tuple(out_names),
            lowering_input_output_aliases=(),
            sim_require_finite=True, sim_require_nnan=True, nc=nc)
        return tuple(outs)

    devices = jax.devices()[:NCORES]
    mesh = Mesh(np.asarray(devices), ("core",))
    sharded = jax.jit(
        shard_map(_body, mesh=mesh,
                  in_specs=(PartitionSpec("core"),) * (n_params + n_outs),
                  out_specs=(PartitionSpec("core"),) * n_outs,
                  check_rep=False),
        donate_argnums=donate, keep_unused=True)
    _cache["runner"] = (sharded, in_names, out_names, out_avals)
    return _cache["runner"]


def kernel(x, adj, W, a):
    import sys
    for p in ("/opt/trn_rl_repo", "/opt/trn_rl_repo/concourse"):
        if p not in sys.path:
            sys.path.insert(0, p)

    x = np.asarray(x)
    adj = np.asarray(adj)
    W = np.ascontiguousarray(np.asarray(W, dtype=np.float32))
    a = np.ascontiguousarray(np.asarray(a, dtype=np.float32))

    xb = np.ascontiguousarray(x).astype(ml_dtypes.bfloat16)
    if adj.dtype == np.int32 and adj.flags.c_contiguous:
        # little-endian low byte of each 0/1 int32 is the value itself
        adjp = np.packbits(adj.view(np.uint8)[:, ::4], axis=1)
    else:
        adjp = np.packbits(adj.astype(np.uint8), axis=1)

    sharded, in_names, out_names, out_avals = _get_runner()
    # Global (concat-across-cores) layout: per-core x_blk / adjp_blk rows
    # concatenate back to exactly xb / adjp — zero copy. Replicated x / W / a
    # tile 8x along axis 0.
    concat = {
        "x": np.tile(xb, (NCORES, 1)),
        "x_blk": xb,
        "adjp_blk": adjp,
        "W": np.tile(W, (NCORES, 1, 1)),
        "a": np.tile(a, (NCORES, 1, 1)),
    }
    concat_in = [concat[name] for name in in_names]
    concat_zeros = [
        np.zeros((NCORES * av.shape[0], *av.shape[1:]), av.dtype)
        for av in out_avals]
    out_arrs = sharded(*concat_in, *concat_zeros)
    out = np.asarray(out_arrs[out_names.index("out_blk")])
    return out.astype(np.float32)


# revision 8
# speedup vs baseline: 11.8511x; 2.4269x over previous
# GAT (graph attention) Trainium2 kernel — 8-core row-parallel SPMD.
#
# Math (per head h, rows I owned by a core):
#   h = x @ W_h ; f1 = h@a1 ; f2 = h@a2 ; z_ij = f1_i + f2_j
#   P_ij = adj_ij ? exp(lrelu(z)) : exp(9e-15 ~= 0) ; att = softmax_j(P)
#   out = elu( (P @ h) / (P @ 1) )
# Device factorization (avoids O(N^2) transcendentals):
#   exp(lrelu(z)) = u'_i * v'_j * max(r_i * w_j, 1)
#     r = e^{0.8 f1}, w = e^{0.8 f2}, u' = e^{0.2 f1}, v' = e^{0.2 f2}
#   E2[j,i] = m^T[j,i] * max(r_i * (w_j v'_j), v'_j)     (ts_dual + tt mult)
#   numer[d,i] = u'_i * ([h|1]^T E2)[d,i] + S[d] - (h^T m^T)[d,i]
#   denom[i]   = u'_i * Y1[i] + N - deg_i
# Transfer-optimized I/O (axon tunnel is ~50 MB/s — e2e is transfer-bound):
#   adj ships bit-packed (np.packbits, 32x smaller); device unpacks via
#   SWDGE u8->bf16 cast load + 8-step is_ge bit-peel (exact in bf16) into
#   the bf16 mask DRAM scratch (with free deg row-sums), then xbar
#   DMA-transpose loads [j-part, i-free] tiles as before.
#   x ships as bf16; out returns as bf16 and is upcast on host.

import numpy as np
import ml_dtypes

N = 8192
EMB = 128
HID = 64
NH = 4
NCORES = 8
NB = N // NCORES  # 1024 rows per core

_cache = {}


def build(n=N, nb=NB):
    import concourse.bass as bass
    import concourse.bacc as bacc
    import concourse.tile as tile
    import concourse.mybir as mybir
    from concourse.masks import make_identity

    fp32 = mybir.dt.float32
    bf16 = mybir.dt.bfloat16
    u8 = mybir.dt.uint8
    Alu = mybir.AluOpType
    Act = mybir.ActivationFunctionType
    MS = bass.MemorySpace

    nbh = nb // 2           # i-half size
    njc = n // 128          # j chunks
    nic = nb // 128         # i chunks (local rows)
    nsub = nbh // 128       # i subchunks per half
    nxc = n // 128          # x chunks

    nc = bacc.Bacc()
    xb_d = nc.declare_dram_parameter("x_blk", [nb, EMB], bf16, isOutput=False)
    adjp_d = nc.declare_dram_parameter("adjp_blk", [nb, n // 8], u8,
                                       isOutput=False)
    W_d = nc.declare_dram_parameter("W", [NH, EMB, HID], fp32, isOutput=False)
    a_d = nc.declare_dram_parameter("a", [NH, 2 * HID, 1], fp32, isOutput=False)
    out_d = nc.declare_dram_parameter("out_blk", [nb, NH * HID], bf16,
                                      isOutput=True)

    with tile.TileContext(nc) as tc:
        with (
            tc.tile_pool(name="const", bufs=1) as const,
            tc.tile_pool(name="ld", bufs=3) as ld,
            tc.tile_pool(name="anat", bufs=2) as anat,
            tc.tile_pool(name="pkp", bufs=1) as pkp,
            tc.tile_pool(name="dramp", bufs=1, space=MS.DRAM) as dramp,
            tc.tile_pool(name="mtp", bufs=6) as mtp,
            tc.tile_pool(name="dep", bufs=8) as dep,
            tc.tile_pool(name="esb", bufs=3) as esb,
        ):
            # ---------------- constants ----------------
            ident = const.tile([128, 128], fp32, name="ident", tag="ident")
            make_identity(nc, ident)
            identb = const.tile([128, 128], bf16, name="identb", tag="identb")
            make_identity(nc, identb)
            ones_row = const.tile([1, 128], fp32, name="ones_row", tag="ones_row")
            nc.vector.memset(ones_row, 1.0)

            madj0 = dramp.tile([nbh, n], bf16, name="madj0", tag="madj0")
            madj1 = dramp.tile([nbh, n], bf16, name="madj1", tag="madj1")
            madj = [madj0, madj1]
            deg_sb = const.tile([128, nic], fp32, name="deg_sb", tag="deg_sb")

            # ------------- all-gather x over the 8 cores -------------
            # Each core ships only its row block; contributions concatenate
            # in device order, reproducing full x in DRAM. Issue first so it
            # overlaps stage A.
            xin = dramp.tile([nb, EMB], bf16, name="xin", tag="xin")
            xg = dramp.tile([n, EMB], bf16, name="xg", tag="xg")
            nc.gpsimd.dma_start(out=xin[:, :], in_=xb_d[:, :])
            nc.gpsimd.collective_compute(
                "AllGather", mybir.AluOpType.bypass,
                replica_groups=[list(range(NCORES))],
                ins=[xin.opt()], outs=[xg.opt()])

            # ------------- stage A: bit-unpack mask (+ deg) -------------
            # adjp rows are packbits(adj_row): byte k bit (7-b) is col 8k+(7-b).
            # Load u8 -> bf16 (values 0..255 exact in bf16), then peel bits
            # MSB-first: bit = (v >= 2^b); v -= 2^b * bit. Strided writes
            # place bit-plane j0=7-b at mask cols j0::8.
            def stageA(ihalf):
                for ics in range(nic // 2):
                    r0 = ihalf * nbh + ics * 128
                    icg = ihalf * (nic // 2) + ics
                    pk = pkp.tile([128, n // 8], bf16, name="pk", tag="pk")
                    nc.gpsimd.dma_start(out=pk, in_=adjp_d[r0:r0 + 128, :])
                    mnat = anat.tile([128, n], bf16, name="mnat", tag="mnat")
                    mv = mnat.rearrange("p (k e) -> p e k", e=8)
                    t0 = pkp.tile([128, n // 8], bf16, name="pt0", tag="pt0")
                    cur, nxt = pk, t0
                    for b in range(7, -1, -1):
                        nc.vector.tensor_scalar(
                            out=mv[:, 7 - b, :], in0=cur,
                            scalar1=float(2 ** b), scalar2=None, op0=Alu.is_ge)
                        if b > 0:
                            nc.vector.scalar_tensor_tensor(
                                out=nxt, in0=mv[:, 7 - b, :],
                                scalar=-float(2 ** b), in1=cur,
                                op0=Alu.mult, op1=Alu.add)
                            cur, nxt = nxt, cur
                    nc.vector.tensor_scalar(
                        out=mnat, in0=mnat, scalar1=1.0, scalar2=None,
                        op0=Alu.mult, op1=Alu.add,
                        accum_out=deg_sb[:, icg:icg + 1])
                    nc.sync.dma_start(
                        out=madj[ihalf][ics * 128:(ics + 1) * 128, :], in_=mnat)

            stageA(0)

            # ---------------- prologue ----------------
            ppsum = tc.alloc_tile_pool(name="ppsum", bufs=2, space=MS.PSUM)
            Wsb = const.tile([128, NH, HID], fp32, name="Wsb", tag="Wsb")
            nc.sync.dma_start(out=Wsb, in_=W_d.rearrange("h e d -> e h d"))
            asb = const.tile([HID, NH, 2], fp32, name="asb", tag="asb")
            nc.sync.dma_start(
                out=asb, in_=a_d.rearrange("h (t d) o -> d h (t o)", t=2))

            # x^T  [128e, n]  (x arrives bf16; transpose via bf16 identity,
            # accumulate/copy to fp32)
            xT = const.tile([128, n], fp32, name="xT", tag="xT")
            for ic in range(nxc):
                xt_nat = ld.tile([128, EMB], bf16, name="xt_nat", tag="xt_nat")
                nc.sync.dma_start(out=xt_nat, in_=xg[ic * 128:(ic + 1) * 128, :])
                ps = ppsum.tile([128, 128], fp32, name="ps", tag="ps")
                nc.tensor.matmul(ps, xt_nat, identb)
                nc.scalar.copy(out=xT[:, ic * 128:(ic + 1) * 128], in_=ps)
            # x_blk^T [128e, nb]
            xbT = const.tile([128, nb], fp32, name="xbT", tag="xbT")
            for ic in range(nic):
                xb_nat = ld.tile([128, EMB], bf16, name="xb_nat", tag="xt_nat")
                nc.sync.dma_start(out=xb_nat, in_=xb_d[ic * 128:(ic + 1) * 128, :])
                ps = ppsum.tile([128, 128], fp32, name="ps", tag="ps")
                nc.tensor.matmul(ps, xb_nat, identb)
                nc.scalar.copy(out=xbT[:, ic * 128:(ic + 1) * 128], in_=ps)

            # xsum[e] = sum_j x[j,e]
            xsum = const.tile([128, 1], fp32, name="xsum", tag="xsum")
            nc.vector.tensor_reduce(xsum, xT, mybir.AxisListType.X, Alu.add)

            # W^T per head; q = [W a1 | W a2] -> Qsb [128e, NH, 2]
            WTsb = const.tile([HID, NH, 128], fp32, name="WTsb", tag="WTsb")
            Qsb = const.tile([128, NH, 2], fp32, name="Qsb", tag="Qsb")
            for h in range(NH):
                wt_ps = ppsum.tile([HID, 128], fp32, name="wt_ps", tag="ps")
                nc.tensor.matmul(wt_ps, Wsb[:, h, :], ident)
                nc.scalar.copy(out=WTsb[:, h, :], in_=wt_ps)
                q_ps = ppsum.tile([128, 2], fp32, name="q_ps", tag="ps")
                nc.tensor.matmul(q_ps, WTsb[:, h, :], asb[:, h, :])
                nc.scalar.copy(out=Qsb[:, h, :], in_=q_ps)

            Qflat = Qsb.rearrange("p h t -> p (h t)")
            Wflat = Wsb.rearrange("e h d -> e (h d)")

            # f columns for all j: Fcol[p, jc, (h t)] = f_{t,h}[jc*128+p]
            Fcol = const.tile([128, njc, 2 * NH], fp32, name="Fcol", tag="Fcol")
            for jc in range(njc):
                f_ps = ppsum.tile([128, 2 * NH], fp32, name="f_ps", tag="ps")
                nc.tensor.matmul(f_ps, xT[:, jc * 128:(jc + 1) * 128], Qflat)
                nc.scalar.copy(out=Fcol[:, jc, :], in_=f_ps)

            # f rows for local block: Frow [8, nb]
            Frow = const.tile([2 * NH, nb], fp32, name="Frow", tag="Frow")
            for half in range(2):
                fr_ps = ppsum.tile([2 * NH, nbh], fp32, name="fr_ps", tag="ps")
                nc.tensor.matmul(fr_ps, Qflat, xbT[:, half * nbh:(half + 1) * nbh])
                nc.scalar.copy(out=Frow[:, half * nbh:(half + 1) * nbh], in_=fr_ps)

            # FrowT [128, nic, 8]
            FrowT = const.tile([128, nic, 2 * NH], fp32, name="FrowT", tag="FrowT")
            for g in range(nic):
                ft_ps = ppsum.tile([128, 2 * NH], fp32, name="ft_ps", tag="ps")
                nc.tensor.matmul(
                    ft_ps, Frow[:, g * 128:(g + 1) * 128],
                    ident[0:2 * NH, 0:2 * NH])
                nc.scalar.copy(out=FrowT[:, g, :], in_=ft_ps)

            # scalar cols (j side): ETc = e^{f2} (= w v'), Vc = e^{0.2 f2}
            ETc = const.tile([128, njc, NH], fp32, name="ETc", tag="ETc")
            Vc = const.tile([128, njc, NH], fp32, name="Vc", tag="Vc")
            for h in range(NH):
                nc.scalar.activation(ETc[:, :, h], Fcol[:, :, 2 * h + 1], Act.Exp)
                nc.scalar.activation(
                    Vc[:, :, h], Fcol[:, :, 2 * h + 1], Act.Exp, scale=0.2)

            # row side: R8 = e^{0.8 Frow}; U'T = e^{0.2 FrowT}
            R8 = const.tile([2 * NH, nb], fp32, name="R8", tag="R8")
            nc.scalar.activation(R8, Frow, Act.Exp, scale=0.8)
            UpT = const.tile([128, nic, 2 * NH], fp32, name="UpT", tag="UpT")
            nc.scalar.activation(
                UpT.rearrange("p a b -> p (a b)"),
                FrowT.rearrange("p a b -> p (a b)"), Act.Exp, scale=0.2)

            # r broadcast per head [128, nb] bf16: bounce rows via DRAM, then
            # broadcast-load with stride-0 partition AP (+ cast) via SWDGE.
            r8_dram = dramp.tile([2 * NH, nb], fp32, name="r8_dram", tag="r8d")
            nc.sync.dma_start(out=r8_dram, in_=R8)
            rbc = []
            for h in range(NH):
                t = const.tile([128, nb], bf16, name=f"rbc{h}", tag=f"rbc{h}")
                srow = r8_dram[2 * h:2 * h + 1, :]
                src_b = bass.AP(
                    tensor=srow.tensor, offset=srow.offset,
                    ap=[[0, 128]] + [list(d) for d in srow.ap[1:]])
                nc.gpsimd.dma_start(out=t, in_=src_b)
                rbc.append(t)

            # H~ [128, njc, NH, HID+1] bf16 (ones col at [.., HID]) for the
            # per-head X passes, plus a contiguous pair layout for hm passes
            # (matmul weights APs must have a single free dimension).
            Hsb = const.tile([128, njc, NH, HID + 1], bf16, name="Hsb", tag="Hsb")
            Hpair = const.tile([128, njc, NH * HID], bf16, name="Hpair", tag="Hpair")
            nc.vector.memset(Hsb[:, :, :, HID], 1.0)
            for jc in range(njc):
                h_ps = ppsum.tile([128, NH, HID], fp32, name="h_ps", tag="ps")
                nc.tensor.matmul(
                    h_ps.rearrange("p h d -> p (h d)"),
                    xT[:, jc * 128:(jc + 1) * 128], Wflat)
                nc.scalar.copy(out=Hsb[:, jc, :, 0:HID], in_=h_ps)
                nc.scalar.copy(
                    out=Hpair[:, jc, :].rearrange("p (h d) -> p h d", h=NH),
                    in_=h_ps)

            # S row then per-head broadcast [128, HID]
            s_ps = ppsum.tile([1, NH * HID], fp32, name="s_ps", tag="ps")
            nc.tensor.matmul(s_ps, xsum, Wflat)
            S_row = const.tile([1, NH * HID], fp32, name="S_row", tag="S_row")
            nc.scalar.copy(out=S_row, in_=s_ps)
            Sb = []
            for h in range(NH):
                sb_ps = ppsum.tile([128, HID], fp32, name="sb_ps", tag="ps")
                nc.tensor.matmul(sb_ps, ones_row, S_row[:, h * HID:(h + 1) * HID])
                t = const.tile([128, HID], fp32, name=f"Sb{h}", tag=f"Sb{h}")
                nc.scalar.copy(out=t, in_=sb_ps)
                Sb.append(t)

            ppsum.release()

            # ------------- stage A part 2, then degbar -------------
            stageA(1)
            degbar = const.tile([128, nic], fp32, name="degbar", tag="degbar")
            nc.vector.tensor_scalar(
                out=degbar, in0=deg_sb, scalar1=-1.0, scalar2=float(n),
                op0=Alu.mult, op1=Alu.add)

            # ---------------- main loop ----------------
            for ihalf in range(2):
                with (
                    tc.tile_pool(name=f"mm{ihalf}", bufs=1, space=MS.PSUM) as mm,
                    tc.tile_pool(name=f"ep{ihalf}", bufs=2, space=MS.PSUM) as ep,
                ):
                    X = [mm.tile([HID + 1, nbh], fp32, name=f"X{h}", tag=f"X{h}")
                         for h in range(NH)]
                    HM = [mm.tile([128, nbh], fp32, name=f"HM{p}", tag=f"HM{p}")
                          for p in range(2)]
                    for jc in range(njc):
                        mT = mtp.tile([128, nbh], bf16, name="mT", tag="mT")
                        nc.sync.dma_start_transpose(
                            out=mT,
                            in_=madj[ihalf][:, jc * 128:(jc + 1) * 128])
                        for h in range(NH):
                            D2 = dep.tile([128, nbh], bf16, name="D2", tag="D2")
                            nc.vector.tensor_scalar(
                                out=D2,
                                in0=rbc[h][:, ihalf * nbh:(ihalf + 1) * nbh],
                                scalar1=ETc[:, jc, h:h + 1],
                                scalar2=Vc[:, jc, h:h + 1],
                                op0=Alu.mult, op1=Alu.max)
                            E2 = dep.tile([128, nbh], bf16, name="E2", tag="E2")
                            eng_tt = nc.gpsimd if h >= 2 else nc.vector
                            eng_tt.tensor_mul(E2, mT, D2)
                            nc.tensor.matmul(
                                X[h], Hsb[:, jc, h, :], E2,
                                start=(jc == 0), stop=(jc == njc - 1))
                        for p in range(2):
                            nc.tensor.matmul(
                                HM[p],
                                Hpair[:, jc, 128 * p:128 * (p + 1)], mT,
                                start=(jc == 0), stop=(jc == njc - 1))

                    # ---------------- epilogue for this half ----------------
                    XS = []
                    for h in range(NH):
                        t = esb.tile([HID + 1, nbh], fp32,
                                     name=f"XS{h}", tag=f"XS{h}", bufs=1)
                        nc.scalar.copy(out=t, in_=X[h])
                        XS.append(t)
                    HMS = []
                    for p in range(2):
                        t = esb.tile([128, nbh], fp32,
                                     name=f"HMS{p}", tag=f"HMS{p}", bufs=1)
                        nc.scalar.copy(out=t, in_=HM[p])
                        HMS.append(t)

                    for isub in range(nsub):
                        g = ihalf * nsub + isub
                        sl = slice(isub * 128, (isub + 1) * 128)
                        hmT = []
                        for p in range(2):
                            tp = ep.tile([128, 128], fp32, name="tp", tag="tp")
                            nc.tensor.matmul(tp, HMS[p][:, sl], ident)
                            t = esb.tile([128, 128], fp32,
                                         name=f"hmT{p}", tag=f"hmT{p}", bufs=2)
                            nc.scalar.copy(out=t, in_=tp)
                            hmT.append(t)
                        out_tile = esb.tile([128, NH * HID], bf16,
                                            name="out_tile", tag="otile", bufs=2)
                        for h in range(NH):
                            tp = ep.tile([128, HID + 1], fp32, name="tpx", tag="tp")
                            nc.tensor.matmul(
                                tp, XS[h][:, sl], ident[0:HID + 1, 0:HID + 1])
                            XT = esb.tile([128, HID + 1], fp32, name="XT", tag="XT")
                            nc.scalar.copy(out=XT, in_=tp)
                            upc = UpT[:, g, 2 * h:2 * h + 1]
                            n1 = esb.tile([128, HID], fp32, name="n1", tag="n1")
                            nc.vector.tensor_scalar(
                                out=n1, in0=XT[:, 0:HID], scalar1=upc,
                                scalar2=None, op0=Alu.mult)
                            n2 = esb.tile([128, HID], fp32, name="n2", tag="n2")
                            nc.vector.scalar_tensor_tensor(
                                out=n2,
                                in0=hmT[h // 2][:, (h % 2) * HID:
                                                (h % 2) * HID + HID],
                                scalar=-1.0, in1=n1, op0=Alu.mult, op1=Alu.add)
                            n3 = esb.tile([128, HID], fp32, name="n3", tag="n3")
                            nc.vector.tensor_add(n3, n2, Sb[h])
                            dcol = esb.tile([128, 1], fp32, name="dcol", tag="dcol")
                            nc.vector.tensor_scalar(
                                out=dcol, in0=XT[:, HID:HID + 1], scalar1=upc,
                                scalar2=degbar[:, g:g + 1],
                                op0=Alu.mult, op1=Alu.add)
                            rec = esb.tile([128, 1], fp32, name="rec", tag="rec")
                            nc.vector.reciprocal(rec, dcol)
                            smT = esb.tile([128, HID], fp32, name="smT", tag="smT")
                            nc.vector.tensor_scalar(
                                out=smT, in0=n3, scalar1=rec, scalar2=None,
                                op0=Alu.mult)
                            # elu = (max(sm,0)-1) + exp(min(sm,0))
                            ea = esb.tile([128, HID], fp32, name="ea", tag="ea")
                            nc.vector.tensor_scalar_min(ea, smT, 0.0)
                            eb = esb.tile([128, HID], fp32, name="eb", tag="eb")
                            nc.scalar.activation(eb, ea, Act.Exp)
                            ec = esb.tile([128, HID], fp32, name="ec", tag="ec")
                            nc.vector.tensor_scalar(
                                out=ec, in0=smT, scalar1=0.0, scalar2=-1.0,
                                op0=Alu.max, op1=Alu.add)
                            nc.vector.tensor_add(
                                out_tile[:, h * HID:(h + 1) * HID], eb, ec)
                        nc.sync.dma_start(
                            out=out_d[g * 128:(g + 1) * 128, :], in_=out_tile)
    nc.compile()
    return nc


def _get_nc():
    if "nc" not in _cache:
        _cache["nc"] = build()
    return _cache["nc"]


def _get_runner():
    # run_bass_via_pjrt rebuilds jax.jit(shard_map(closure)) per call, so the
    # jit cache misses every time (multi-second retrace+compile). Build the
    # jitted sharded callable ONCE and reuse it.
    if "runner" in _cache:
        return _cache["runner"]

    import jax
    from jax.sharding import Mesh, PartitionSpec
    from jax.experimental.shard_map import shard_map
    import concourse.mybir as mybir
    from concourse.bass2jax import (
        _bass_exec_p, partition_id_tensor, install_neuronx_cc_hook)

    nc = _get_nc()
    install_neuronx_cc_hook()
    partition_name = (
        nc.partition_id_tensor.name if nc.partition_id_tensor else None)
    in_names, out_names, out_avals = [], [], []
    for alloc in nc.m.functions[0].allocations:
        if not isinstance(alloc, mybir.MemoryLocationSet):
            continue
        name = alloc.memorylocations[0].name
        if alloc.kind == "ExternalInput":
            if name != partition_name:
                in_names.append(name)
        elif alloc.kind == "ExternalOutput":
            out_names.append(name)
            out_avals.append(jax.core.ShapedArray(
                tuple(alloc.tensor_shape), mybir.dt.np(alloc.dtype)))
    n_params = len(in_names)
    n_outs = len(out_avals)
    all_names = in_names + out_names
    if partition_name is not None:
        all_names = all_names + [partition_name]
    donate = tuple(range(n_params, n_params + n_outs))

    def _body(*args):
        operands = list(args)
        if partition_name is not None:
            operands.append(partition_id_tensor())
        outs = _bass_exec_p.bind(
            *operands, out_avals=tuple(out_avals),
            in_names=tuple(all_names), out_names=tuple(out_names),
            lowering_input_output_aliases=(),
            sim_require_finite=True, sim_require_nnan=True, nc=nc)
        return tuple(outs)

    devices = jax.devices()[:NCORES]
    mesh = Mesh(np.asarray(devices), ("core",))
    sharded = jax.jit(
        shard_map(_body, mesh=mesh,
                  in_specs=(PartitionSpec("core"),) * (n_params + n_outs),
                  out_specs=(PartitionSpec("core"),) * n_outs,
                  check_rep=False),
        donate_argnums=donate, keep_unused=True)
    _cache["runner"] = (sharded, in_names, out_names, out_avals)
    return _cache["runner"]


def kernel(x, adj, W, a):
    import sys
    for p in ("/opt/trn_rl_repo", "/opt/trn_rl_repo/concourse"):
        if p not in sys.path:
            sys.path.insert(0, p)

    x = np.asarray(x)
    adj = np.asarray(adj)
    W = np.ascontiguousarray(np.asarray(W, dtype=np.float32))
    a = np.ascontiguousarray(np.asarray(a, dtype=np.float32))

    xb = np.ascontiguousarray(x).astype(ml_dtypes.bfloat16)
    if adj.dtype == np.int32 and adj.flags.c_contiguous:
        # little-endian low byte of each 0/1 int32 is the value itself
        adjp = np.packbits(adj.view(np.uint8)[:, ::4], axis=1)
    else:
        adjp = np.packbits(adj.astype(np.uint8), axis=1)

    sharded, in_names, out_names, out_avals = _get_runner()
    # Global (concat-across-cores) layout: per-core x_blk / adjp_blk rows
    # concatenate back to exactly xb / adjp — zero copy. Replicated x / W / a
    # tile 8x along axis 0.
    concat = {
        "x_blk": xb,
        "adjp_blk": adjp,
        "W": np.tile(W, (NCORES, 1, 1)),
        "a": np.tile(a, (NCORES, 1, 1)),
    }
    concat_in = [concat[name] for name in in_names]
    concat_zeros = [
        np.zeros((NCORES * av.shape[0], *av.shape[1:]), av.dtype)
        for av in out_avals]
    out_arrs = sharded(*concat_in, *concat_zeros)
    out = np.asarray(out_arrs[out_names.index("out_blk")])
    return out.astype(np.float32)


# revision 10
# speedup vs baseline: 28.1046x; 2.3715x over previous
# GAT (graph attention) Trainium2 kernel — 8-core row-parallel SPMD.
#
# Math (per head h, rows I owned by a core):
#   h = x @ W_h ; f1 = h@a1 ; f2 = h@a2 ; z_ij = f1_i + f2_j
#   P_ij = adj_ij ? exp(lrelu(z)) : exp(9e-15 ~= 0) ; att = softmax_j(P)
#   out = elu( (P @ h) / (P @ 1) )
# Device factorization (avoids O(N^2) transcendentals):
#   exp(lrelu(z)) = u'_i * v'_j * max(r_i * w_j, 1)
#     r = e^{0.8 f1}, w = e^{0.8 f2}, u' = e^{0.2 f1}, v' = e^{0.2 f2}
#   E2[j,i] = m^T[j,i] * max(r_i * (w_j v'_j), v'_j)     (ts_dual + tt mult)
#   numer[d,i] = u'_i * ([h|1]^T E2)[d,i] + S[d] - (h^T m^T)[d,i]
#   denom[i]   = u'_i * Y1[i] + N - deg_i
# Transfer-optimized I/O (axon tunnel is ~50 MB/s — e2e is transfer-bound):
#   adj ships bit-packed (np.packbits, 32x smaller); device unpacks via
#   SWDGE u8->bf16 cast load + 8-step is_ge bit-peel (exact in bf16) into
#   the bf16 mask DRAM scratch (with free deg row-sums), then xbar
#   DMA-transpose loads [j-part, i-free] tiles as before.
#   x ships as bf16; out returns as bf16 and is upcast on host.

import numpy as np
import ml_dtypes

N = 8192
EMB = 128
HID = 64
NH = 4
NCORES = 8
NB = N // NCORES  # 1024 rows per core

_cache = {}


def build(n=N, nb=NB):
    import concourse.bass as bass
    import concourse.bacc as bacc
    import concourse.tile as tile
    import concourse.mybir as mybir
    from concourse.masks import make_identity

    fp32 = mybir.dt.float32
    bf16 = mybir.dt.bfloat16
    u8 = mybir.dt.uint8
    Alu = mybir.AluOpType
    Act = mybir.ActivationFunctionType
    MS = bass.MemorySpace

    nbh = nb // 2           # i-half size
    njc = n // 128          # j chunks
    nic = nb // 128         # i chunks (local rows)
    nsub = nbh // 128       # i subchunks per half
    nxc = n // 128          # x chunks

    nc = bacc.Bacc()
    xb_d = nc.declare_dram_parameter("x_blk", [nb, EMB], bf16, isOutput=False)
    adjp_d = nc.declare_dram_parameter("adjp_blk", [nb, n // 8], u8,
                                       isOutput=False)
    W_d = nc.declare_dram_parameter("W", [NH, EMB, HID], fp32, isOutput=False)
    a_d = nc.declare_dram_parameter("a", [NH, 2 * HID, 1], fp32, isOutput=False)
    out_d = nc.declare_dram_parameter("out_blk", [nb, NH * HID], bf16,
                                      isOutput=True)

    with tile.TileContext(nc) as tc:
        with (
            tc.tile_pool(name="const", bufs=1) as const,
            tc.tile_pool(name="ld", bufs=3) as ld,
            tc.tile_pool(name="anat", bufs=2) as anat,
            tc.tile_pool(name="pkp", bufs=1) as pkp,
            tc.tile_pool(name="dramp", bufs=1, space=MS.DRAM) as dramp,
            tc.tile_pool(name="mtp", bufs=6) as mtp,
            tc.tile_pool(name="dep", bufs=8) as dep,
            tc.tile_pool(name="esb", bufs=3) as esb,
        ):
            # ---------------- constants ----------------
            ident = const.tile([128, 128], fp32, name="ident", tag="ident")
            make_identity(nc, ident)
            identb = const.tile([128, 128], bf16, name="identb", tag="identb")
            make_identity(nc, identb)
            ones_row = const.tile([1, 128], fp32, name="ones_row", tag="ones_row")
            nc.vector.memset(ones_row, 1.0)

            madj0 = dramp.tile([nbh, n], bf16, name="madj0", tag="madj0")
            madj1 = dramp.tile([nbh, n], bf16, name="madj1", tag="madj1")
            madj = [madj0, madj1]
            deg_sb = const.tile([128, nic], fp32, name="deg_sb", tag="deg_sb")

            # ------------- all-gather x over the 8 cores -------------
            # Each core ships only its row block; contributions concatenate
            # in device order, reproducing full x in DRAM. Issue first so it
            # overlaps stage A.
            xin = dramp.tile([nb, EMB], bf16, name="xin", tag="xin")
            xg = dramp.tile([n, EMB], bf16, name="xg", tag="xg")
            nc.gpsimd.dma_start(out=xin[:, :], in_=xb_d[:, :])
            nc.gpsimd.collective_compute(
                "AllGather", mybir.AluOpType.bypass,
                replica_groups=[list(range(NCORES))],
                ins=[xin.opt()], outs=[xg.opt()])

            # ------------- stage A: bit-unpack mask (+ deg) -------------
            # adjp rows are packbits(adj_row): byte k bit (7-b) is col 8k+(7-b).
            # Load u8 -> bf16 (values 0..255 exact in bf16), then peel bits
            # MSB-first: bit = (v >= 2^b); v -= 2^b * bit. Strided writes
            # place bit-plane j0=7-b at mask cols j0::8.
            def stageA(ihalf):
                for ics in range(nic // 2):
                    r0 = ihalf * nbh + ics * 128
                    icg = ihalf * (nic // 2) + ics
                    pk = pkp.tile([128, n // 8], bf16, name="pk", tag="pk")
                    nc.gpsimd.dma_start(out=pk, in_=adjp_d[r0:r0 + 128, :])
                    mnat = anat.tile([128, n], bf16, name="mnat", tag="mnat")
                    mv = mnat.rearrange("p (k e) -> p e k", e=8)
                    t0 = pkp.tile([128, n // 8], bf16, name="pt0", tag="pt0")
                    cur, nxt = pk, t0
                    for b in range(7, -1, -1):
                        nc.vector.tensor_scalar(
                            out=mv[:, 7 - b, :], in0=cur,
                            scalar1=float(2 ** b), scalar2=None, op0=Alu.is_ge)
                        if b > 0:
                            nc.vector.scalar_tensor_tensor(
                                out=nxt, in0=mv[:, 7 - b, :],
                                scalar=-float(2 ** b), in1=cur,
                                op0=Alu.mult, op1=Alu.add)
                            cur, nxt = nxt, cur
                    nc.vector.tensor_scalar(
                        out=mnat, in0=mnat, scalar1=1.0, scalar2=None,
                        op0=Alu.mult, op1=Alu.add,
                        accum_out=deg_sb[:, icg:icg + 1])
                    nc.sync.dma_start(
                        out=madj[ihalf][ics * 128:(ics + 1) * 128, :], in_=mnat)

            stageA(0)

            # ---------------- prologue ----------------
            ppsum = tc.alloc_tile_pool(name="ppsum", bufs=2, space=MS.PSUM)
            Wsb = const.tile([128, NH, HID], fp32, name="Wsb", tag="Wsb")
            nc.sync.dma_start(out=Wsb, in_=W_d.rearrange("h e d -> e h d"))
            asb = const.tile([HID, NH, 2], fp32, name="asb", tag="asb")
            nc.sync.dma_start(
                out=asb, in_=a_d.rearrange("h (t d) o -> d h (t o)", t=2))

            # x^T  [128e, n]  (x arrives bf16; transpose via bf16 identity,
            # accumulate/copy to fp32)
            xT = const.tile([128, n], fp32, name="xT", tag="xT")
            for ic in range(nxc):
                xt_nat = ld.tile([128, EMB], bf16, name="xt_nat", tag="xt_nat")
                nc.sync.dma_start(out=xt_nat, in_=xg[ic * 128:(ic + 1) * 128, :])
                ps = ppsum.tile([128, 128], fp32, name="ps", tag="ps")
                nc.tensor.matmul(ps, xt_nat, identb)
                nc.scalar.copy(out=xT[:, ic * 128:(ic + 1) * 128], in_=ps)
            # x_blk^T [128e, nb]
            xbT = const.tile([128, nb], fp32, name="xbT", tag="xbT")
            for ic in range(nic):
                xb_nat = ld.tile([128, EMB], bf16, name="xb_nat", tag="xt_nat")
                nc.sync.dma_start(out=xb_nat, in_=xb_d[ic * 128:(ic + 1) * 128, :])
                ps = ppsum.tile([128, 128], fp32, name="ps", tag="ps")
                nc.tensor.matmul(ps, xb_nat, identb)
                nc.scalar.copy(out=xbT[:, ic * 128:(ic + 1) * 128], in_=ps)

            # xsum[e] = sum_j x[j,e]
            xsum = const.tile([128, 1], fp32, name="xsum", tag="xsum")
            nc.vector.tensor_reduce(xsum, xT, mybir.AxisListType.X, Alu.add)

            # W^T per head; q = [W a1 | W a2] -> Qsb [128e, NH, 2]
            WTsb = const.tile([HID, NH, 128], fp32, name="WTsb", tag="WTsb")
            Qsb = const.tile([128, NH, 2], fp32, name="Qsb", tag="Qsb")
            for h in range(NH):
                wt_ps = ppsum.tile([HID, 128], fp32, name="wt_ps", tag="ps")
                nc.tensor.matmul(wt_ps, Wsb[:, h, :], ident)
                nc.scalar.copy(out=WTsb[:, h, :], in_=wt_ps)
                q_ps = ppsum.tile([128, 2], fp32, name="q_ps", tag="ps")
                nc.tensor.matmul(q_ps, WTsb[:, h, :], asb[:, h, :])
                nc.scalar.copy(out=Qsb[:, h, :], in_=q_ps)

            Qflat = Qsb.rearrange("p h t -> p (h t)")
            Wflat = Wsb.rearrange("e h d -> e (h d)")

            # f columns for all j: Fcol[p, jc, (h t)] = f_{t,h}[jc*128+p]
            Fcol = const.tile([128, njc, 2 * NH], fp32, name="Fcol", tag="Fcol")
            for jc in range(njc):
                f_ps = ppsum.tile([128, 2 * NH], fp32, name="f_ps", tag="ps")
                nc.tensor.matmul(f_ps, xT[:, jc * 128:(jc + 1) * 128], Qflat)
                nc.scalar.copy(out=Fcol[:, jc, :], in_=f_ps)

            # f rows for local block: Frow [8, nb]
            Frow = const.tile([2 * NH, nb], fp32, name="Frow", tag="Frow")
            for half in range(2):
                fr_ps = ppsum.tile([2 * NH, nbh], fp32, name="fr_ps", tag="ps")
                nc.tensor.matmul(fr_ps, Qflat, xbT[:, half * nbh:(half + 1) * nbh])
                nc.scalar.copy(out=Frow[:, half * nbh:(half + 1) * nbh], in_=fr_ps)

            # FrowT [128, nic, 8]
            FrowT = const.tile([128, nic, 2 * NH], fp32, name="FrowT", tag="FrowT")
            for g in range(nic):
                ft_ps = ppsum.tile([128, 2 * NH], fp32, name="ft_ps", tag="ps")
                nc.tensor.matmul(
                    ft_ps, Frow[:, g * 128:(g + 1) * 128],
                    ident[0:2 * NH, 0:2 * NH])
                nc.scalar.copy(out=FrowT[:, g, :], in_=ft_ps)

            # scalar cols (j side): ETc = e^{f2} (= w v'), Vc = e^{0.2 f2}
            ETc = const.tile([128, njc, NH], fp32, name="ETc", tag="ETc")
            Vc = const.tile([128, njc, NH], fp32, name="Vc", tag="Vc")
            for h in range(NH):
                nc.scalar.activation(ETc[:, :, h], Fcol[:, :, 2 * h + 1], Act.Exp)
                nc.scalar.activation(
                    Vc[:, :, h], Fcol[:, :, 2 * h + 1], Act.Exp, scale=0.2)

            # row side: R8 = e^{0.8 Frow}; U'T = e^{0.2 FrowT}
            R8 = const.tile([2 * NH, nb], fp32, name="R8", tag="R8")
            nc.scalar.activation(R8, Frow, Act.Exp, scale=0.8)
            UpT = const.tile([128, nic, 2 * NH], fp32, name="UpT", tag="UpT")
            nc.scalar.activation(
                UpT.rearrange("p a b -> p (a b)"),
                FrowT.rearrange("p a b -> p (a b)"), Act.Exp, scale=0.2)

            # r broadcast per head [128, nb] bf16: bounce rows via DRAM, then
            # broadcast-load with stride-0 partition AP (+ cast) via SWDGE.
            r8_dram = dramp.tile([2 * NH, nb], fp32, name="r8_dram", tag="r8d")
            nc.sync.dma_start(out=r8_dram, in_=R8)
            rbc = []
            for h in range(NH):
                t = const.tile([128, nb], bf16, name=f"rbc{h}", tag=f"rbc{h}")
                srow = r8_dram[2 * h:2 * h + 1, :]
                src_b = bass.AP(
                    tensor=srow.tensor, offset=srow.offset,
                    ap=[[0, 128]] + [list(d) for d in srow.ap[1:]])
                nc.gpsimd.dma_start(out=t, in_=src_b)
                rbc.append(t)

            # H~ [128, njc, NH, HID+1] bf16 (ones col at [.., HID]) for the
            # per-head X passes, plus a contiguous pair layout for hm passes
            # (matmul weights APs must have a single free dimension).
            Hsb = const.tile([128, njc, NH, HID + 1], bf16, name="Hsb", tag="Hsb")
            Hpair = const.tile([128, njc, NH * HID], bf16, name="Hpair", tag="Hpair")
            nc.vector.memset(Hsb[:, :, :, HID], 1.0)
            for jc in range(njc):
                h_ps = ppsum.tile([128, NH, HID], fp32, name="h_ps", tag="ps")
                nc.tensor.matmul(
                    h_ps.rearrange("p h d -> p (h d)"),
                    xT[:, jc * 128:(jc + 1) * 128], Wflat)
                nc.scalar.copy(out=Hsb[:, jc, :, 0:HID], in_=h_ps)
                nc.scalar.copy(
                    out=Hpair[:, jc, :].rearrange("p (h d) -> p h d", h=NH),
                    in_=h_ps)

            # S row then per-head broadcast [128, HID]
            s_ps = ppsum.tile([1, NH * HID], fp32, name="s_ps", tag="ps")
            nc.tensor.matmul(s_ps, xsum, Wflat)
            S_row = const.tile([1, NH * HID], fp32, name="S_row", tag="S_row")
            nc.scalar.copy(out=S_row, in_=s_ps)
            Sb = []
            for h in range(NH):
                sb_ps = ppsum.tile([128, HID], fp32, name="sb_ps", tag="ps")
                nc.tensor.matmul(sb_ps, ones_row, S_row[:, h * HID:(h + 1) * HID])
                t = const.tile([128, HID], fp32, name=f"Sb{h}", tag=f"Sb{h}")
                nc.scalar.copy(out=t, in_=sb_ps)
                Sb.append(t)

            ppsum.release()

            # ------------- stage A part 2, then degbar -------------
            stageA(1)
            degbar = const.tile([128, nic], fp32, name="degbar", tag="degbar")
            nc.vector.tensor_scalar(
                out=degbar, in0=deg_sb, scalar1=-1.0, scalar2=float(n),
                op0=Alu.mult, op1=Alu.add)

            # ---------------- main loop ----------------
            for ihalf in range(2):
                with (
                    tc.tile_pool(name=f"mm{ihalf}", bufs=1, space=MS.PSUM) as mm,
                    tc.tile_pool(name=f"ep{ihalf}", bufs=2, space=MS.PSUM) as ep,
                ):
                    X = [mm.tile([HID + 1, nbh], fp32, name=f"X{h}", tag=f"X{h}")
                         for h in range(NH)]
                    HM = [mm.tile([128, nbh], fp32, name=f"HM{p}", tag=f"HM{p}")
                          for p in range(2)]
                    for jc in range(njc):
                        mT = mtp.tile([128, nbh], bf16, name="mT", tag="mT")
                        nc.sync.dma_start_transpose(
                            out=mT,
                            in_=madj[ihalf][:, jc * 128:(jc + 1) * 128])
                        for h in range(NH):
                            D2 = dep.tile([128, nbh], bf16, name="D2", tag="D2")
                            nc.vector.tensor_scalar(
                                out=D2,
                                in0=rbc[h][:, ihalf * nbh:(ihalf + 1) * nbh],
                                scalar1=ETc[:, jc, h:h + 1],
                                scalar2=Vc[:, jc, h:h + 1],
                                op0=Alu.mult, op1=Alu.max)
                            E2 = dep.tile([128, nbh], bf16, name="E2", tag="E2")
                            eng_tt = nc.gpsimd if h >= 2 else nc.vector
                            eng_tt.tensor_mul(E2, mT, D2)
                            nc.tensor.matmul(
                                X[h], Hsb[:, jc, h, :], E2,
                                start=(jc == 0), stop=(jc == njc - 1))
                        for p in range(2):
                            nc.tensor.matmul(
                                HM[p],
                                Hpair[:, jc, 128 * p:128 * (p + 1)], mT,
                                start=(jc == 0), stop=(jc == njc - 1))

                    # ---------------- epilogue for this half ----------------
                    XS = []
                    for h in range(NH):
                        t = esb.tile([HID + 1, nbh], fp32,
                                     name=f"XS{h}", tag=f"XS{h}", bufs=1)
                        nc.scalar.copy(out=t, in_=X[h])
                        XS.append(t)
                    HMS = []
                    for p in range(2):
                        t = esb.tile([128, nbh], fp32,
                                     name=f"HMS{p}", tag=f"HMS{p}", bufs=1)
                        nc.scalar.copy(out=t, in_=HM[p])
                        HMS.append(t)

                    for isub in range(nsub):
                        g = ihalf * nsub + isub
                        sl = slice(isub * 128, (isub + 1) * 128)
                        hmT = []
                        for p in range(2):
                            tp = ep.tile([128, 128], fp32, name="tp", tag="tp")
                            nc.tensor.matmul(tp, HMS[p][:, sl], ident)
                            t = esb.tile([128, 128], fp32,
                                         name=f"hmT{p}", tag=f"hmT{p}", bufs=2)
                            nc.scalar.copy(out=t, in_=tp)
                            hmT.append(t)
                        out_tile = esb.tile([128, NH * HID], bf16,
                                            name="out_tile", tag="otile", bufs=2)
                        for h in range(NH):
                            tp = ep.tile([128, HID + 1], fp32, name="tpx", tag="tp")
                            nc.tensor.matmul(
                                tp, XS[h][:, sl], ident[0:HID + 1, 0:HID + 1])
                            XT = esb.tile([128, HID + 1], fp32, name="XT", tag="XT")
                            nc.scalar.copy(out=XT, in_=tp)
                            upc = UpT[:, g, 2 * h:2 * h + 1]
                            n1 = esb.tile([128, HID], fp32, name="n1", tag="n1")
                            nc.vector.tensor_scalar(
                                out=n1, in0=XT[:, 0:HID], scalar1=upc,
                                scalar2=None, op0=Alu.mult)
                            n2 = esb.tile([128, HID], fp32, name="n2", tag="n2")
                            nc.vector.scalar_tensor_tensor(
                                out=n2,
                                in0=hmT[h // 2][:, (h % 2) * HID:
                                                (h % 2) * HID + HID],
                                scalar=-1.0, in1=n1, op0=Alu.mult, op1=Alu.add)
                            n3 = esb.tile([128, HID], fp32, name="n3", tag="n3")
                            nc.vector.tensor_add(n3, n2, Sb[h])
                            dcol = esb.tile([128, 1], fp32, name="dcol", tag="dcol")
                            nc.vector.tensor_scalar(
                                out=dcol, in0=XT[:, HID:HID + 1], scalar1=upc,
                                scalar2=degbar[:, g:g + 1],
                                op0=Alu.mult, op1=Alu.add)
                            rec = esb.tile([128, 1], fp32, name="rec", tag="rec")
                            nc.vector.reciprocal(rec, dcol)
                            smT = esb.tile([128, HID], fp32, name="smT", tag="smT")
                            nc.vector.tensor_scalar(
                                out=smT, in0=n3, scalar1=rec, scalar2=None,
                                op0=Alu.mult)
                            # elu = (max(sm,0)-1) + exp(min(sm,0))
                            ea = esb.tile([128, HID], fp32, name="ea", tag="ea")
                            nc.vector.tensor_scalar_min(ea, smT, 0.0)
                            eb = esb.tile([128, HID], fp32, name="eb", tag="eb")
                            nc.scalar.activation(eb, ea, Act.Exp)
                            ec = esb.tile([128, HID], fp32, name="ec", tag="ec")
                            nc.vector.tensor_scalar(
                                out=ec, in0=smT, scalar1=0.0, scalar2=-1.0,
                                op0=Alu.max, op1=Alu.add)
                            nc.vector.tensor_add(
                                out_tile[:, h * HID:(h + 1) * HID], eb, ec)
                        nc.sync.dma_start(
                            out=out_d[g * 128:(g + 1) * 128, :], in_=out_tile)
    nc.compile()
    return nc


def _get_nc():
    if "nc" not in _cache:
        _cache["nc"] = build()
    return _cache["nc"]


def _get_runner():
    # run_bass_via_pjrt rebuilds jax.jit(shard_map(closure)) per call, so the
    # jit cache misses every time (multi-second retrace+compile). Build the
    # jitted sharded callable ONCE and reuse it.
    if "runner" in _cache:
        return _cache["runner"]

    import functools
    import jax
    import jax.numpy as jnp
    from jax.sharding import Mesh, PartitionSpec, NamedSharding
    from jax.experimental.shard_map import shard_map
    import concourse.mybir as mybir
    from concourse.bass2jax import (
        _bass_exec_p, partition_id_tensor, install_neuronx_cc_hook)

    nc = _get_nc()
    install_neuronx_cc_hook()
    partition_name = (
        nc.partition_id_tensor.name if nc.partition_id_tensor else None)
    in_names, out_names, out_avals = [], [], []
    for alloc in nc.m.functions[0].allocations:
        if not isinstance(alloc, mybir.MemoryLocationSet):
            continue
        name = alloc.memorylocations[0].name
        if alloc.kind == "ExternalInput":
            if name != partition_name:
                in_names.append(name)
        elif alloc.kind == "ExternalOutput":
            out_names.append(name)
            out_avals.append(jax.core.ShapedArray(
                tuple(alloc.tensor_shape), mybir.dt.np(alloc.dtype)))
    n_params = len(in_names)
    n_outs = len(out_avals)
    all_names = in_names + out_names
    if partition_name is not None:
        all_names = all_names + [partition_name]
    donate = tuple(range(n_params, n_params + n_outs))

    def _body(*args):
        operands = list(args)
        if partition_name is not None:
            operands.append(partition_id_tensor())
        outs = _bass_exec_p.bind(
            *operands, out_avals=tuple(out_avals),
            in_names=tuple(all_names), out_names=tuple(out_names),
            lowering_input_output_aliases=(),
            sim_require_finite=True, sim_require_nnan=True, nc=nc)
        return tuple(outs)

    devices = jax.devices()[:NCORES]
    mesh = Mesh(np.asarray(devices), ("core",))
    shard = NamedSharding(mesh, PartitionSpec("core"))
    sharded = jax.jit(
        shard_map(_body, mesh=mesh,
                  in_specs=(PartitionSpec("core"),) * (n_params + n_outs),
                  out_specs=(PartitionSpec("core"),) * n_outs,
                  check_rep=False),
        donate_argnums=donate, keep_unused=True)

    # donated output buffers created on-device (no host->device zeros ship)
    zshapes = [((NCORES * av.shape[0],) + tuple(av.shape[1:]), av.dtype)
               for av in out_avals]
    make_zeros = jax.jit(
        lambda: tuple(jnp.zeros(s, d) for s, d in zshapes),
        out_shardings=(shard,) * len(zshapes))

    # bit-pack adj on the multithreaded XLA CPU backend (3x numpy packbits)
    w8 = (np.uint8(1) << np.arange(7, -1, -1, dtype=np.uint8))

    @functools.partial(jax.jit, backend="cpu")
    def pack_cpu(adj32):
        r = adj32.astype(jnp.uint8).reshape(N, N // 8, 8)
        return (r * w8).sum(axis=-1, dtype=jnp.uint8)

    _cache["runner"] = (
        sharded, in_names, out_names, out_avals, shard, make_zeros, pack_cpu)
    return _cache["runner"]


def kernel(x, adj, W, a):
    import sys
    for p in ("/opt/trn_rl_repo", "/opt/trn_rl_repo/concourse"):
        if p not in sys.path:
            sys.path.insert(0, p)
    import jax

    x = np.asarray(x)
    adj = np.asarray(adj)
    W = np.asarray(W, dtype=np.float32)
    a = np.asarray(a, dtype=np.float32)

    (sharded, in_names, out_names, out_avals, shard, make_zeros,
     pack_cpu) = _get_runner()

    # Stage small inputs + donated zeros first (async), then bit-pack adj on
    # the CPU backend and ship it as one sharded put (row-blocks land on
    # their owning cores directly).
    xb = np.ascontiguousarray(x).astype(ml_dtypes.bfloat16)
    named = {
        "x_blk": jax.device_put(xb, shard),
        "W": jax.device_put(np.tile(W, (NCORES, 1, 1)), shard),
        "a": jax.device_put(np.tile(a, (NCORES, 1, 1)), shard),
    }
    zs = make_zeros()
    adjp = np.asarray(pack_cpu(adj))
    named["adjp_blk"] = jax.device_put(adjp, shard)

    out_arrs = sharded(*[named[n_] for n_ in in_names], *zs)
    ob = out_arrs[out_names.index("out_blk")]

    # async per-shard fetch, assemble + upcast
    parts = sorted(ob.addressable_shards, key=lambda s_: s_.index[0].start)
    datas = [p.data for p in parts]
    for d in datas:
        d.copy_to_host_async()
    res = np.empty((N, NH * HID), np.float32)
    for p_, d in zip(parts, datas):
        res[p_.index] = np.asarray(d)
    return res
